# revision 1
# baseline (speedup 1.0000x reference)
"""GPDconv (GNN message passing) Trainium2 Bass kernel — sorted-grid design.

Batch-parallel over 8 NeuronCores (one batch per core). dma_scatter_add on
TRN2 loses colliding read-modify-write updates, so both segment-sums are
restructured as host-sorted fixed-capacity rank grids:

  sigma1 (targets = edge_Gauss, NUM_PTS): edges sorted by target into regions
    (R x COLS x rank_base). Slot values come from a dma_gather of node
    pair-rows (x+grid+grid_weight fp16, pair elements so indices fit int16)
    plus a dma_gather of a small per-p table (rnorm, base_weight) produced by
    a dense normalization pass. Region 0 reduces in-partition to dense x_hat
    rows; overflow regions reduce then scatter-add with distinct targets
    (collision-free; pad columns aimed at distinct cold targets with zero
    values).
  phase C: y = (x_hat @ W) * D^T reduced over KM via PE.
  sigma2 (targets = edge_grid>>1 node pairs, N/2): same machinery; values are
    gauss * y[edge_Gauss] with parity masks picking the 32-float half of the
    64-wide pair row.

Host does index/layout prep only (sorting, ranks, packing, int16 wrapping)
plus the final reshape/transpose.
"""
import sys
from math import exp, sqrt

if '/opt/trn_rl_repo' not in sys.path:
    sys.path.insert(0, '/opt/trn_rl_repo')

import numpy as np
import concourse.bacc as bacc
import concourse.mybir as mybir
import concourse.tile as tile
from concourse import bass_utils, library_config, masks

f32 = mybir.dt.float32
f16 = mybir.dt.float16
i16 = mybir.dt.int16

CFG_FULL = dict(N=65536, NUM_PTS=4096, K=32, CIN=32, COUT=32, KM=16)
CFG_SMALL = dict(N=2048, NUM_PTS=512, K=8, CIN=32, COUT=32, KM=16)


def _pois_sf(k, lam):
    term = exp(-lam)
    cdf = term
    for i in range(1, k + 1):
        term *= lam / i
        cdf += term
    return max(0.0, 1.0 - cdf)


def _cap6(ntgt, lam, k):
    p = _pois_sf(k, lam)
    m = ntgt * p
    c = m + 6.0 * sqrt(max(1.0, ntgt * p * (1 - p))) + 64
    c = min(ntgt, c)
    return max(128, int(-(-c // 128)) * 128)


def make_regions(lam, ntgt):
    """[(R, COLS, rank_base), ...] — region 0 covers every target densely."""
    if lam >= 8:
        return [(lam, ntgt, 0),
                (4, _cap6(ntgt, lam, lam), lam),
                (12, _cap6(ntgt, lam, lam + 4), lam + 4),
                ((3 * lam) // 2, 128, lam + 16)]
    return [(4, ntgt, 0),
            (2, _cap6(ntgt, 4, 4), 4),
            (4, _cap6(ntgt, 4, 6), 6),
            (8, _cap6(ntgt, 4, 10), 10),
            (16, 128, 18)]


def chunk_list(regs):
    """Deterministic chunking shared by host packing and device build:
    returns [(slot_base, num_slots)] per chunk."""
    out = []
    base = 0
    for R, C, rb in regs:
        MO = C // 128
        moc = max(1, 8192 // (R * 128))
        for c0 in range(0, MO, moc):
            mo_n = min(moc, MO - c0)
            out.append((base + c0 * R * 128, mo_n * R * 128))
        base += R * C
    return out


def pack_tab_chunks(tab, regs):
    """(S, T) slot-major table -> [128, sum(T*Jc)] per-chunk transposed."""
    T = tab.shape[1]
    blocks = []
    for sbase, S in chunk_list(regs):
        blk = tab[sbase:sbase + S].reshape(S // 128, 128, T).transpose(1, 2, 0)
        blocks.append(blk.reshape(128, T * (S // 128)))
    return np.ascontiguousarray(np.concatenate(blocks, axis=1))


def assign_slots(tgt, regs, ntgt):
    """Returns (slot_of_edge, total_slots, [col->target per overflow region])."""
    E = len(tgt)
    order = np.argsort(tgt, kind='stable')
    cnt = np.bincount(tgt, minlength=ntgt)
    starts = np.concatenate([[0], np.cumsum(cnt)])[:-1]
    rank = np.empty(E, np.int64)
    rank[order] = np.arange(E) - np.repeat(starts, cnt)
    max_rank = sum(r[0] for r in regs)
    assert cnt.max() <= max_rank, (cnt.max(), max_rank)
    slot = np.full(E, -1, np.int64)
    bases = np.cumsum([0] + [R * C for R, C, _ in regs])
    scat_tgts = []
    for ri, (R, C, rb) in enumerate(regs):
        sel = (rank >= rb) & (rank < rb + R)
        if ri == 0:
            cols = tgt[sel]
        else:
            hot = np.nonzero(cnt > rb)[0]
            assert len(hot) <= C, (ri, len(hot), C)
            col_of = np.full(ntgt, -1, np.int64)
            col_of[hot] = np.arange(len(hot))
            cols = col_of[tgt[sel]]
            # pad columns -> distinct cold targets (zero values, race-free)
            cold = np.nonzero(cnt <= rb)[0]
            t = np.empty(C, np.int64)
            t[:len(hot)] = hot
            t[len(hot):] = cold[:C - len(hot)]
            scat_tgts.append(t)
        r = rank[sel] - rb
        slot[sel] = bases[ri] + (cols // 128) * (R * 128) + r * 128 + (cols % 128)
    assert (slot >= 0).all()
    return slot, int(bases[-1]), scat_tgts


def _wrap16(a):
    return np.ascontiguousarray(np.tile(a.reshape(-1, 16).T, (8, 1)))


def host_prep(cfg, x_b, grid_b, gw_b, eg_b, ega_b, basepts, base_weight, D, weights):
    N, NUM_PTS, K = cfg["N"], cfg["NUM_PTS"], cfg["K"]
    CIN, COUT, KM = cfg["CIN"], cfg["COUT"], cfg["KM"]
    E = K * NUM_PTS
    PCOLS = NUM_PTS // 128
    eg = eg_b.T.reshape(-1).astype(np.int64)        # (E,) [k, p] order
    ega = ega_b.T.reshape(-1).astype(np.int64)
    pp = np.tile(np.arange(NUM_PTS), K)

    regs1 = make_regions(K, NUM_PTS)
    slot1, S1T, sc1 = assign_slots(ega, regs1, NUM_PTS)
    s1xi = np.zeros(S1T, np.int16)
    s1xi[slot1] = (eg >> 1).astype(np.int16)
    s1ri = np.zeros(S1T, np.int16)
    s1ri[slot1] = pp.astype(np.int16)
    tab1 = np.zeros((S1T, 4), np.float16)
    tab1[slot1, 0] = (1 - (eg & 1)).astype(np.float16)
    tab1[slot1, 1] = (eg & 1).astype(np.float16)
    tab1[slot1, 2] = basepts[ega, 0].astype(np.float16)
    tab1[slot1, 3] = basepts[ega, 1].astype(np.float16)

    m2 = eg >> 1
    regs2 = make_regions(4, N // 2)
    slot2, S2T, sc2 = assign_slots(m2, regs2, N // 2)
    s2yi = np.zeros(S2T, np.int16)
    s2yi[slot2] = ega.astype(np.int16)
    tab2 = np.zeros((S2T, 8), np.float16)
    tab2[slot2, 0] = grid_b[eg, 0].astype(np.float16)
    tab2[slot2, 1] = grid_b[eg, 1].astype(np.float16)
    tab2[slot2, 2] = basepts[ega, 0].astype(np.float16)
    tab2[slot2, 3] = basepts[ega, 1].astype(np.float16)
    tab2[slot2, 4] = base_weight[pp, 0].astype(np.float16)
    tab2[slot2, 5] = base_weight[pp, 1].astype(np.float16)
    tab2[slot2, 6] = (1 - (eg & 1)).astype(np.float16)
    tab2[slot2, 7] = (eg & 1).astype(np.float16)

    s1sc = _wrap16(np.concatenate(sc1).astype(np.int16))
    s2sc = _wrap16(np.concatenate(sc2).astype(np.int16))

    def lay_dense(v):
        return np.ascontiguousarray(
            v.reshape(K, PCOLS, 128).transpose(2, 1, 0).reshape(128, E // 128))
    dtab = np.stack([
        lay_dense(grid_b[eg, 0].reshape(K, NUM_PTS)),
        lay_dense(grid_b[eg, 1].reshape(K, NUM_PTS)),
        lay_dense(gw_b[eg].reshape(K, NUM_PTS)),
        lay_dense(basepts[ega, 0].reshape(K, NUM_PTS)),
        lay_dense(basepts[ega, 1].reshape(K, NUM_PTS)),
    ], axis=-1).astype(np.float16)
    bwd = np.stack([base_weight[:, 0].reshape(PCOLS, 128).T,
                    base_weight[:, 1].reshape(PCOLS, 128).T], axis=-1)

    rows = np.zeros((N, 64), np.float32)
    rows[:, :CIN] = x_b.T
    rows[:, CIN] = grid_b[:, 0]
    rows[:, CIN + 1] = grid_b[:, 1]
    rows[:, CIN + 2] = gw_b
    return dict(
        xcat=rows.astype(np.float16).reshape(N // 2, 128),
        s1xi=_wrap16(s1xi), s1ri=_wrap16(s1ri),
        s1tab=pack_tab_chunks(tab1, regs1),
        s1sc=s1sc,
        s2yi=_wrap16(s2yi),
        s2tab=pack_tab_chunks(tab2, regs2),
        s2sc=s2sc,
        dtab=dtab,
        bwd=np.ascontiguousarray(bwd.astype(np.float32)),
        wfl=np.ascontiguousarray(weights.reshape(CIN, COUT * KM).astype(np.float32)),
        dt_t=np.ascontiguousarray(D.T.astype(np.float32)),
    )


def build(nc, cfg):
    N, NUM_PTS, K = cfg["N"], cfg["NUM_PTS"], cfg["K"]
    CIN, COUT, KM = cfg["CIN"], cfg["COUT"], cfg["KM"]
    E = K * NUM_PTS
    PCOLS = NUM_PTS // 128
    TT = NUM_PTS // 128
    OJ = COUT * KM
    STAGE = cfg.get("STAGE", 99)
    regs1 = make_regions(K, NUM_PTS)
    regs2 = make_regions(4, N // 2)
    S1T = sum(R * C for R, C, _ in regs1)
    S2T = sum(R * C for R, C, _ in regs2)
    SC1 = sum(C for R, C, _ in regs1[1:])
    SC2 = sum(C for R, C, _ in regs2[1:])

    xcat_d = nc.dram_tensor("xcat", [N // 2, 128], f16, kind="ExternalInput")
    s1xi_d = nc.dram_tensor("s1xi", [128, S1T // 16], i16, kind="ExternalInput")
    s1ri_d = nc.dram_tensor("s1ri", [128, S1T // 16], i16, kind="ExternalInput")
    s1tab_d = nc.dram_tensor("s1tab", [128, (S1T // 128) * 4], f16, kind="ExternalInput")
    s1sc_d = nc.dram_tensor("s1sc", [128, SC1 // 16], i16, kind="ExternalInput")
    s2yi_d = nc.dram_tensor("s2yi", [128, S2T // 16], i16, kind="ExternalInput")
    s2tab_d = nc.dram_tensor("s2tab", [128, (S2T // 128) * 8], f16, kind="ExternalInput")
    s2sc_d = nc.dram_tensor("s2sc", [128, SC2 // 16], i16, kind="ExternalInput")
    dtab_d = nc.dram_tensor("dtab", [128, E // 128, 5], f16, kind="ExternalInput")
    bwd_d = nc.dram_tensor("bwd", [128, PCOLS, 2], f32, kind="ExternalInput")
    wfl_d = nc.dram_tensor("wfl", [CIN, OJ], f32, kind="ExternalInput")
    dtt_d = nc.dram_tensor("dt_t", [NUM_PTS, KM], f32, kind="ExternalInput")
    out_d = nc.dram_tensor("out", [N // 2 + 128, 64], f32, kind="ExternalOutput")

    xhat_d = nc.dram_tensor("xhat_tbl", [NUM_PTS + 128, 64], f32, kind="Internal")
    ycat_d = nc.dram_tensor("ycat_tbl", [NUM_PTS, 64], f32, kind="Internal")
    rncat_d = nc.dram_tensor("rncat_tbl", [NUM_PTS, 128], f16, kind="Internal")

    mult, add, subtract = (mybir.AluOpType.mult, mybir.AluOpType.add,
                           mybir.AluOpType.subtract)
    Exp = mybir.ActivationFunctionType.Exp
    X = mybir.AxisListType.X

    with tile.TileContext(nc) as tc:
        with tc.tile_pool(name="consts", bufs=1) as cp:
            ident = cp.tile([128, 128], f32)
            masks.make_identity(nc, ident[:])
            nc.gpsimd.load_library(library_config.mlp)

            wfl = cp.tile([CIN, OJ], f32)
            nc.sync.dma_start(wfl[:], wfl_d[:])
            bwd = cp.tile([128, PCOLS * 2], f32)
            bwd3 = bwd[:].rearrange("p (q t) -> p q t", t=2)
            nc.sync.dma_start(bwd3, bwd_d[:])
            rn_sb = cp.tile([128, PCOLS * 128], f16)
            rn3 = rn_sb[:].rearrange("p (q c) -> p q c", c=128)

            # ---------- dense pass: rnorm per p -> rncat table ----------
            with tc.tile_pool(name="dense", bufs=1) as dp:
                JD = E // 128
                dtab = dp.tile([128, JD * 5], f16)
                dt3 = dtab[:].rearrange("p (j t) -> p j t", t=5)
                nc.sync.dma_start(dt3, dtab_d[:])
                dd0 = dp.tile([128, JD], f32)
                dd1 = dp.tile([128, JD], f32)
                nc.vector.tensor_tensor(dd0[:], dt3[:, :, 0], dt3[:, :, 3], op=subtract)
                nc.vector.tensor_tensor(dd0[:], dd0[:], dd0[:], op=mult)
                nc.vector.tensor_tensor(dd1[:], dt3[:, :, 1], dt3[:, :, 4], op=subtract)
                nc.vector.tensor_tensor(dd1[:], dd1[:], dd1[:], op=mult)
                d0k = dd0[:].rearrange("p (q k) -> p q k", k=K)
                d1k = dd1[:].rearrange("p (q k) -> p q k", k=K)
                nc.vector.tensor_tensor(d0k, d0k,
                                        bwd3[:, :, 0].broadcast_to((128, PCOLS, K)),
                                        op=mult)
                nc.vector.tensor_tensor(d1k, d1k,
                                        bwd3[:, :, 1].broadcast_to((128, PCOLS, K)),
                                        op=mult)
                nc.vector.tensor_tensor(dd0[:], dd0[:], dd1[:], op=add)
                du = dp.tile([128, JD], f32)
                nc.scalar.activation(du[:], dd0[:], Exp, scale=-1.0)
                nc.vector.tensor_tensor(du[:], du[:], dt3[:, :, 2], op=mult)
                nc.vector.tensor_tensor(du[:], du[:], du[:], op=mult)
                nsq = dp.tile([128, PCOLS], f32)
                nc.vector.reduce_sum(nsq[:].unsqueeze(2),
                                     du[:].rearrange("p (q k) -> p q k", k=K), axis=X)
                nc.scalar.activation(nsq[:], nsq[:],
                                     mybir.ActivationFunctionType.Sqrt)
                nc.vector.tensor_scalar_add(nsq[:], nsq[:], 1e-5)
                nc.vector.reciprocal(nsq[:], nsq[:])
                nc.vector.memset(rn_sb[:], 0.0)
                nc.vector.tensor_copy(rn3[:, :, 0], nsq[:])
                nc.vector.tensor_copy(rn3[:, :, 1], bwd3[:, :, 0])
                nc.vector.tensor_copy(rn3[:, :, 2], bwd3[:, :, 1])
                nc.sync.dma_start(
                    rncat_d.ap().rearrange("(q p) c -> p q c", p=128), rn3)

            # ---------- sigma1 -> x_hat ----------
            xh_stage = [cp.tile([128, (C // 128) * CIN], f32, tag=f"xhs{ri}",
                                name=f"xhs{ri}")
                        for ri, (R, C, rb) in enumerate(regs1[1:])]
            s1sc_sb = cp.tile([128, SC1 // 16], i16)
            nc.sync.dma_start(s1sc_sb[:], s1sc_d[:])
            with tc.tile_pool(name="ph1", bufs=2) as p1:
                base = 0
                for ri, (R, C, rb) in enumerate(regs1 if STAGE >= 2 else []):
                    MO = C // 128
                    moc = max(1, 8192 // (R * 128))
                    for c0 in range(0, MO, moc):
                        mo_n = min(moc, MO - c0)
                        S = mo_n * R * 128
                        J = S // 128
                        sbase = base + c0 * R * 128
                        isl = slice(sbase // 16, (sbase + S) // 16)
                        jsl = slice(sbase // 128, (sbase + S) // 128)

                        xi = p1.tile([128, 512], i16, tag="xi")
                        nc.sync.dma_start(xi[:, :S // 16], s1xi_d[:, isl])
                        rix = p1.tile([128, 512], i16, tag="rix")
                        nc.sync.dma_start(rix[:, :S // 16], s1ri_d[:, isl])
                        tb = p1.tile([128, 4 * 64], f16, tag="tb")
                        nc.sync.dma_start(tb[:, :4 * J],
                                          s1tab_d[:, 4 * (sbase // 128):
                                                  4 * (sbase // 128) + 4 * J])
                        tbT = tb[:, :4 * J].rearrange("p (t j) -> p t j", j=J)

                        gx = p1.tile([128, 64 * 128], f16, tag="gx", bufs=3)
                        gx3 = gx[:].rearrange("p (j e) -> p j e", e=128)
                        nc.gpsimd.dma_gather(gx3[:, :J, :], xcat_d[:],
                                             xi[:, :S // 16], S, S, 128,
                                             elem_step=128, single_packet=False)
                        rn = p1.tile([128, 64 * 128], f16, tag="rn")
                        rg3 = rn[:].rearrange("p (j e) -> p j e", e=128)
                        nc.gpsimd.dma_gather(rg3[:, :J, :], rncat_d[:],
                                             rix[:, :S // 16], S, S, 128,
                                             elem_step=128, single_packet=False)

                        mev = tbT[:, 0, :]
                        md = tbT[:, 1, :]
                        rnf = p1.tile([128, 3 * 64], f32, tag="rnf")
                        rnfT = rnf[:].rearrange("p (t j) -> p t j", j=64)
                        nc.vector.tensor_copy(
                            rnfT[:, :, :J],
                            rg3[:, :J, 0:3].rearrange("p j t -> p t j"))
                        gf = p1.tile([128, 3 * 64], f32, tag="gf")
                        gfT = gf[:].rearrange("p (t j) -> p t j", j=64)
                        tf = p1.tile([128, 3 * 64], f32, tag="tf")
                        tfT = tf[:].rearrange("p (t j) -> p t j", j=64)
                        nc.vector.tensor_tensor(
                            gfT[:, :, :J],
                            gx3[:, :J, 32:35].rearrange("p j t -> p t j"),
                            mev.unsqueeze(1).broadcast_to((128, 3, J)), op=mult)
                        nc.vector.tensor_tensor(
                            tfT[:, :, :J],
                            gx3[:, :J, 96:99].rearrange("p j t -> p t j"),
                            md.unsqueeze(1).broadcast_to((128, 3, J)), op=mult)
                        nc.vector.tensor_tensor(gfT[:, :, :J], gfT[:, :, :J],
                                                tfT[:, :, :J], op=add)
                        dd = p1.tile([128, 2 * 64], f32, tag="dd")
                        ddT = dd[:].rearrange("p (t j) -> p t j", j=64)
                        nc.vector.tensor_tensor(ddT[:, :, :J], gfT[:, 0:2, :J],
                                                tbT[:, 2:4, :], op=subtract)
                        nc.vector.tensor_tensor(ddT[:, :, :J], ddT[:, :, :J],
                                                ddT[:, :, :J], op=mult)
                        nc.vector.tensor_tensor(ddT[:, :, :J], ddT[:, :, :J],
                                                rnfT[:, 1:3, :J], op=mult)
                        ga = p1.tile([128, 64], f32, tag="ga")
                        nc.vector.tensor_tensor(ga[:, :J], ddT[:, 0, :J],
                                                ddT[:, 1, :J], op=add)
                        nc.scalar.activation(ga[:, :J], ga[:, :J], Exp, scale=-1.0)
                        nc.vector.tensor_tensor(ga[:, :J], ga[:, :J],
                                                gfT[:, 2, :J], op=mult)
                        nc.vector.tensor_tensor(ga[:, :J], ga[:, :J],
                                                rnfT[:, 0, :J], op=mult)
                        wlo = p1.tile([128, 64], f32, tag="wlo")
                        whi = p1.tile([128, 64], f32, tag="whi")
                        nc.vector.tensor_tensor(wlo[:, :J], ga[:, :J], mev, op=mult)
                        nc.vector.tensor_tensor(whi[:, :J], ga[:, :J], md, op=mult)
                        v1 = p1.tile([128, 64 * CIN], f32, tag="v1")
                        v13 = v1[:].rearrange("p (j e) -> p j e", e=CIN)
                        t1 = p1.tile([128, 64 * CIN], f32, tag="t1")
                        t13 = t1[:].rearrange("p (j e) -> p j e", e=CIN)
                        nc.vector.tensor_tensor(
                            v13[:, :J, :], gx3[:, :J, 0:CIN],
                            wlo[:, :J].broadcast_to((128, J, CIN)), op=mult)
                        nc.vector.tensor_tensor(
                            t13[:, :J, :], gx3[:, :J, 64:64 + CIN],
                            whi[:, :J].broadcast_to((128, J, CIN)), op=mult)
                        nc.vector.tensor_tensor(v13[:, :J, :], v13[:, :J, :],
                                                t13[:, :J, :], op=add)
                        vr = v1[:, :J * CIN].rearrange(
                            "p (mo r e) -> p mo e r", r=R, e=CIN)
                        if ri == 0:
                            red = p1.tile([128, 8 * CIN], f32, tag="red")
                            red3 = red[:].rearrange("p (mo e) -> p mo e", e=CIN)
                            nc.vector.reduce_sum(red3[:, :mo_n, :].unsqueeze(3),
                                                 vr, axis=X)
                            nc.sync.dma_start(
                                xhat_d.ap()[c0 * 128:(c0 + mo_n) * 128, 0:CIN]
                                .rearrange("(mo p) e -> p mo e", p=128),
                                red3[:, :mo_n, :])
                        else:
                            st3 = xh_stage[ri - 1][:].rearrange(
                                "p (mo e) -> p mo e", e=CIN)
                            nc.vector.reduce_sum(
                                st3[:, c0:c0 + mo_n, :].unsqueeze(3), vr, axis=X)
                    base += R * C
                scb = 0
                for ri, (R, C, rb) in enumerate(regs1[1:] if STAGE >= 2 else []):
                    st3 = xh_stage[ri][:].rearrange("p (mo e) -> p mo e", e=CIN)
                    for q0 in range(0, C, 4096):
                        qn = min(4096, C - q0)
                        nc.gpsimd.dma_scatter_add(
                            xhat_d[:, 0:CIN], st3[:, q0 // 128:(q0 + qn) // 128, :],
                            s1sc_sb[:, (scb + q0) // 16:(scb + q0 + qn) // 16],
                            qn, qn, CIN, elem_step=64, single_packet=False)
                    scb += C

            # ---------- phase C ----------
            ycat_sb = cp.tile([128, TT * 64], f32)
            with tc.tile_pool(name="phc", bufs=2) as pc, \
                    tc.tile_pool(name="psum", bufs=2, space="PSUM") as pq:
                for t in range(TT if STAGE >= 3 else 0):
                    xh = pc.tile([128, CIN], f32)
                    nc.sync.dma_start(xh[:], xhat_d[t * 128:(t + 1) * 128, 0:CIN])
                    xhtp = pq.tile([CIN, 128], f32)
                    nc.tensor.transpose(xhtp[:], xh[:], ident[:])
                    xht = pc.tile([CIN, 128], f32)
                    nc.vector.tensor_copy(xht[:], xhtp[:])
                    o1p = pq.tile([128, OJ], f32)
                    nc.tensor.matmul(o1p[:], xht[:], wfl[:])
                    dtt = pc.tile([128, KM], f32)
                    nc.sync.dma_start(dtt[:], dtt_d[t * 128:(t + 1) * 128, :])
                    o1 = pc.tile([128, OJ], f32)
                    nc.vector.tensor_tensor(
                        o1[:].rearrange("p (o j) -> p o j", j=KM),
                        o1p[:].rearrange("p (o j) -> p o j", j=KM),
                        dtt[:].unsqueeze(1).broadcast_to((128, COUT, KM)), op=mult)
                    ysb3 = ycat_sb[:].rearrange("p (t c) -> p t c", c=64)
                    nc.vector.reduce_sum(
                        ysb3[:, t, 0:COUT].unsqueeze(2),
                        o1[:].rearrange("p (o j) -> p o j", j=KM), axis=X)
                    nc.vector.tensor_copy(ysb3[:, t, 32:32 + COUT],
                                          ysb3[:, t, 0:COUT])
            if STAGE >= 3:
                nc.sync.dma_start(
                    ycat_d.ap().rearrange("(t p) c -> p t c", p=128),
                    ycat_sb[:].rearrange("p (t c) -> p t c", c=64))

            # ---------- sigma2 -> out ----------
            o_stage = [cp.tile([128, (C // 128) * 64], f32, tag=f"os{ri}",
                               name=f"os{ri}")
                       for ri, (R, C, rb) in enumerate(regs2[1:])]
            s2sc_sb = cp.tile([128, SC2 // 16], i16)
            nc.sync.dma_start(s2sc_sb[:], s2sc_d[:])
            with tc.tile_pool(name="ph2", bufs=2) as p2:
                base = 0
                for ri, (R, C, rb) in enumerate(regs2 if STAGE >= 4 else []):
                    MO = C // 128
                    moc = max(1, 8192 // (R * 128))
                    for c0 in range(0, MO, moc):
                        mo_n = min(moc, MO - c0)
                        S = mo_n * R * 128
                        J = S // 128
                        sbase = base + c0 * R * 128
                        isl = slice(sbase // 16, (sbase + S) // 16)
                        jsl = slice(sbase // 128, (sbase + S) // 128)

                        yi = p2.tile([128, 512], i16, tag="yi")
                        nc.sync.dma_start(yi[:, :S // 16], s2yi_d[:, isl])
                        tb = p2.tile([128, 8 * 64], f16, tag="tb2")
                        nc.sync.dma_start(tb[:, :8 * J],
                                          s2tab_d[:, 8 * (sbase // 128):
                                                  8 * (sbase // 128) + 8 * J])
                        tbT = tb[:, :8 * J].rearrange("p (t j) -> p t j", j=J)
                        gy = p2.tile([128, 64 * 64], f32, tag="gy", bufs=3)
                        gy3 = gy[:].rearrange("p (j e) -> p j e", e=64)
                        nc.gpsimd.dma_gather(gy3[:, :J, :], ycat_d[:],
                                             yi[:, :S // 16], S, S, 64,
                                             elem_step=64, single_packet=False)
                        dd = p2.tile([128, 2 * 64], f32, tag="ddb")
                        ddT = dd[:].rearrange("p (t j) -> p t j", j=64)
                        nc.vector.tensor_tensor(ddT[:, :, :J], tbT[:, 0:2, :],
                                                tbT[:, 2:4, :], op=subtract)
                        nc.vector.tensor_tensor(ddT[:, :, :J], ddT[:, :, :J],
                                                ddT[:, :, :J], op=mult)
                        nc.vector.tensor_tensor(ddT[:, :, :J], ddT[:, :, :J],
                                                tbT[:, 4:6, :], op=mult)
                        ga = p2.tile([128, 64], f32, tag="gab")
                        nc.vector.tensor_tensor(ga[:, :J], ddT[:, 0, :J],
                                                ddT[:, 1, :J], op=add)
                        nc.scalar.activation(ga[:, :J], ga[:, :J], Exp, scale=-1.0)
                        gm = p2.tile([128, 64 * 2], f32, tag="gm")
                        gm3 = gm[:].rearrange("p (j h) -> p j h", h=2)
                        nc.vector.tensor_tensor(gm3[:, :J, 0], ga[:, :J],
                                                tbT[:, 6, :], op=mult)
                        nc.vector.tensor_tensor(gm3[:, :J, 1], ga[:, :J],
                                                tbT[:, 7, :], op=mult)
                        v2 = p2.tile([128, 64 * 64], f32, tag="v2")
                        v24 = v2[:].rearrange("p (j h e) -> p j h e", h=2, e=32)
                        nc.vector.tensor_tensor(
                            v24[:, :J, :, :],
                            gy3[:, :J, :].rearrange("p j (h e) -> p j h e", h=2),
                            gm3[:, :J, :].unsqueeze(3).broadcast_to((128, J, 2, 32)),
                            op=mult)
                        vr = v2[:, :J * 64].rearrange(
                            "p (mo r e) -> p mo e r", r=R, e=64)
                        if ri == 0:
                            red = p2.tile([128, 16 * 64], f32, tag="red2")
                            red3 = red[:].rearrange("p (mo e) -> p mo e", e=64)
                            nc.vector.reduce_sum(red3[:, :mo_n, :].unsqueeze(3),
                                                 vr, axis=X)
                            nc.sync.dma_start(
                                out_d.ap()[c0 * 128:(c0 + mo_n) * 128, :]
                                .rearrange("(mo p) e -> p mo e", p=128),
                                red3[:, :mo_n, :])
                        else:
                            st3 = o_stage[ri - 1][:].rearrange(
                                "p (mo e) -> p mo e", e=64)
                            nc.vector.reduce_sum(
                                st3[:, c0:c0 + mo_n, :].unsqueeze(3), vr, axis=X)
                    base += R * C
                scb = 0
                for ri, (R, C, rb) in enumerate(regs2[1:] if STAGE >= 5 else []):
                    st3 = o_stage[ri][:].rearrange("p (mo e) -> p mo e", e=64)
                    for q0 in range(0, C, 4096):
                        qn = min(4096, C - q0)
                        nc.gpsimd.dma_scatter_add(
                            out_d[:], st3[:, q0 // 128:(q0 + qn) // 128, :],
                            s2sc_sb[:, (scb + q0) // 16:(scb + q0 + qn) // 16],
                            qn, qn, 64, elem_step=64, single_packet=False)
                    scb += C
    return nc


def make_in_maps(cfg, x, grid, grid_weight, edge_grid, edge_Gauss, basepts,
                 base_weight, D, weights):
    return [host_prep(cfg, x[b], grid[b], grid_weight[b], edge_grid[b],
                      edge_Gauss[b], basepts, base_weight, D, weights)
            for b in range(x.shape[0])]


def finish(cfg, out_tbl):
    return np.ascontiguousarray(
        out_tbl[:cfg["N"] // 2].reshape(cfg["N"], 32)[:, :cfg["COUT"]].T)


_BUILT = {}


def _get_nc(cfg_key="full"):
    if cfg_key not in _BUILT:
        cfg = CFG_FULL if cfg_key == "full" else CFG_SMALL
        nc = bacc.Bacc("TRN2", target_bir_lowering=False,
                       dynamic_dma_scratch_size=32768)
        build(nc, cfg)
        nc.compile()
        _BUILT[cfg_key] = nc
    return _BUILT[cfg_key]


def kernel(x, grid, grid_weight, edge_grid, edge_Gauss, basepts, base_weight,
           D, weights, _trace=False):
    cfg = CFG_FULL
    x = np.asarray(x)
    in_maps = make_in_maps(cfg, np.asarray(x, np.float32), np.asarray(grid),
                           np.asarray(grid_weight), np.asarray(edge_grid),
                           np.asarray(edge_Gauss), np.asarray(basepts),
                           np.asarray(base_weight), np.asarray(D),
                           np.asarray(weights))
    nc = _get_nc("full")
    res = bass_utils.run_bass_kernel_spmd(
        nc, in_maps, core_ids=list(range(x.shape[0])), trace=_trace)
    out = np.stack([finish(cfg, res.results[b]["out"])
                    for b in range(x.shape[0])])
    kernel.last_result = res
    return out



# revision 11
# speedup vs baseline: 2.0844x; 2.0844x over previous
"""GPDconv (GNN message passing) Trainium2 Bass kernel — PE one-hot design.

Batch-parallel over 8 NeuronCores (one batch per core). The previous design
spent ~4ms/core in Q7 SWDGE descriptor generation (~8ns per gather index,
~500k indices). This version keeps exactly TWO per-edge SWDGE passes (the
provable floor) and does all aggregation on the PE via one-hot matmuls:

  sigma1: edges sorted into 32 host-balanced target-blocks (128 ega-targets,
    exactly 4096 edges each). One dma_gather of x pair-rows per edge
    (+ ~6% slot padding from the rnorm partition constraint). Per 128-edge
    group: V1 = u*rnorm*x_row, one-hot over within-block target -> PE matmul
    accumulating x_hat^T [32ch, 128t] in PSUM. rnorm[p] is delivered by a
    96-plane select: edge partition q == (p + rot_c) % 128 for one of three
    rotations (3-choice load balancing), rnorm planes live at [q, 32c+j].
  C: y = (x_hat @ W) . D^T per 128-target tile (targets in permuted order).
  sigma2: edges sorted into 256 host-balanced pair-blocks (128 node-pairs,
    exactly 512 edges each). One dma_gather of y rows per edge (zero pad).
    V2 = gauss*(parity masks)*y, one-hot over within-block pair -> PE matmul
    -> out pair-rows [128, 64] per block, written permuted; host unpermutes.

Host does index/layout prep only (sorting, balancing, packing, int16);
all value math (gauss, norms, products, reductions) runs on device.
"""
import sys

if '/opt/trn_rl_repo' not in sys.path:
    sys.path.insert(0, '/opt/trn_rl_repo')

import numpy as np
import concourse.bacc as bacc
import concourse.mybir as mybir
import concourse.tile as tile
from concourse import bass_utils, library_config, masks

f32 = mybir.dt.float32
f16 = mybir.dt.float16
i16 = mybir.dt.int16

CFG = dict(N=65536, NUM_PTS=4096, K=32, CIN=32, COUT=32, KM=16,
           G1FIX=34, ROTS=(0, 43, 86), S2CHUNK=8)

mult, add, subtract = (mybir.AluOpType.mult, mybir.AluOpType.add,
                       mybir.AluOpType.subtract)
is_equal = mybir.AluOpType.is_equal
Exp = mybir.ActivationFunctionType.Exp
X = mybir.AxisListType.X


def _wrap16(a):
    return np.ascontiguousarray(np.tile(a.reshape(-1, 16).T, (8, 1)))


def _balance_blocks(deg, nblocks, per_block_items, per_block_sum):
    """Partition items into nblocks of exactly per_block_items items with
    degree sums exactly per_block_sum. Snake-deal + exact swap repair."""
    deg = np.asarray(deg, np.int64)
    n = len(deg)
    assert n == nblocks * per_block_items
    assert deg.sum() == nblocks * per_block_sum
    order = np.argsort(-deg, kind='stable')
    # snake deal: rows of nblocks, alternating direction
    rows = order.reshape(per_block_items, nblocks)
    for r in range(1, per_block_items, 2):
        rows[r] = rows[r][::-1]
    blocks = [list(rows[:, b]) for b in range(nblocks)]
    sums = np.array([deg[b].sum() for b in blocks], np.int64)
    for _ in range(100000):
        dev = sums - per_block_sum
        if not dev.any():
            break
        hi = int(np.argmax(dev))
        lo = int(np.argmin(dev))
        dstar = int(min(dev[hi], -dev[lo]))
        ha = np.asarray(blocks[hi])
        la = np.asarray(blocks[lo])
        da, db = deg[ha], deg[la]
        ua = np.unique(da)
        ub = np.unique(db)
        found = None
        for want in range(dstar, 0, -1):
            hit = ua[np.isin(ua - want, ub)]
            if len(hit):
                va = int(hit[0])
                ai = int(np.nonzero(da == va)[0][0])
                bj = int(np.nonzero(db == va - want)[0][0])
                found = (ai, bj, want)
                break
        assert found is not None, (dev[hi], dev[lo], ua, ub)
        ai, bj, want = found
        a_it, b_it = int(ha[ai]), int(la[bj])
        blocks[hi][ai] = b_it
        blocks[lo][bj] = a_it
        sums[hi] -= want
        sums[lo] += want
    assert (sums == per_block_sum).all(), sums
    return [np.asarray(b, np.int64) for b in blocks]


def _assign_bins(res, rots, cap):
    """3-choice capacitated assignment: edge i may go to bin
    (res[i]+rot)%128; return bin per edge with loads <= cap.
    Greedy lightest-bin init + BFS augmenting-path eviction."""
    n = len(res)
    nr = len(rots)
    cands = np.stack([(res + r) % 128 for r in rots], 1)   # (n, nr)
    cnt = np.zeros(128, np.int64)
    choice = np.zeros(n, np.int64)
    order = np.random.default_rng(0).permutation(n)
    for i in order:
        c = cands[i]
        j = int(np.argmin(cnt[c]))
        choice[i] = j
        cnt[c[j]] += 1
    # bin -> member edge list
    members = [[] for _ in range(128)]
    for i in range(n):
        members[int(cands[i, choice[i]])].append(i)
    while True:
        over = [b for b in range(128) if cnt[b] > cap]
        if not over:
            break
        s = over[0]
        # BFS from s to any bin with load < cap via edge reassignments
        parent = {s: None}
        frontier = [s]
        goal = None
        while frontier and goal is None:
            nxt = []
            for u in frontier:
                for i in members[u]:
                    for j in range(nr):
                        v = int(cands[i, j])
                        if v == u or v in parent:
                            continue
                        parent[v] = (u, i, j)
                        if cnt[v] < cap:
                            goal = v
                            break
                        nxt.append(v)
                    if goal is not None:
                        break
                if goal is not None:
                    break
            frontier = nxt
        assert goal is not None, "no augmenting path; raise G1FIX"
        # walk back, reassigning one edge per hop
        v = goal
        while parent[v] is not None:
            u, i, j = parent[v]
            members[u].remove(i)
            members[v].append(i)
            choice[i] = j
            cnt[u] -= 1
            cnt[v] += 1
            v = u
    assert cnt.max() <= cap, (cnt.max(), cap)
    return cands[np.arange(n), choice]


def host_prep(cfg, x_b, grid_b, gw_b, eg_b, ega_b, basepts, base_weight, D,
              weights):
    N, NUM_PTS, K = cfg["N"], cfg["NUM_PTS"], cfg["K"]
    CIN, COUT, KM = cfg["CIN"], cfg["COUT"], cfg["KM"]
    G1FIX, ROTS = cfg["G1FIX"], cfg["ROTS"]
    E = K * NUM_PTS
    PCOLS = NUM_PTS // 128
    eg = eg_b.T.reshape(-1).astype(np.int64)        # (E,) [k, p] order
    ega = ega_b.T.reshape(-1).astype(np.int64)
    pp = np.tile(np.arange(NUM_PTS), K)

    # ---------------- xcat pair-row table ----------------
    rows = np.zeros((N, 64), np.float32)
    rows[:, :CIN] = x_b.T
    rows[:, CIN] = grid_b[:, 0]
    rows[:, CIN + 1] = grid_b[:, 1]
    rows[:, CIN + 2] = gw_b
    xcat = rows.astype(np.float16).reshape(N // 2, 128)

    # ---------------- dense tab (rnorm pass) ----------------
    def lay_dense(v):
        return np.ascontiguousarray(
            v.reshape(K, PCOLS, 128).transpose(2, 1, 0).reshape(128, E // 128))
    dtab = np.stack([
        lay_dense(grid_b[eg, 0].reshape(K, NUM_PTS)),
        lay_dense(grid_b[eg, 1].reshape(K, NUM_PTS)),
        lay_dense(gw_b[eg].reshape(K, NUM_PTS)),
        lay_dense(basepts[ega, 0].reshape(K, NUM_PTS)),
        lay_dense(basepts[ega, 1].reshape(K, NUM_PTS)),
    ], axis=-1).astype(np.float16)
    bwd = np.stack([base_weight[:, 0].reshape(PCOLS, 128).T,
                    base_weight[:, 1].reshape(PCOLS, 128).T], axis=-1)

    # ---------------- sigma1: balanced target blocks ----------------
    tdeg = np.bincount(ega, minlength=NUM_PTS)
    blocks1 = _balance_blocks(tdeg, 32, 128, E // 32)
    t_newrow = np.empty(NUM_PTS, np.int64)          # orig target -> new row
    t_local = np.empty(NUM_PTS, np.int64)
    t_block = np.empty(NUM_PTS, np.int64)
    for b in range(32):
        t_newrow[blocks1[b]] = 128 * b + np.arange(128)
        t_local[blocks1[b]] = np.arange(128)
        t_block[blocks1[b]] = b

    SG1 = 32 * G1FIX
    S1 = SG1 * 128
    xidx1 = np.zeros(S1, np.int16)
    tab1 = np.zeros((S1, 8), np.float16)            # bpx bpy bwx bwy me mo egar prow
    tab1[:, 6] = -1.0
    tab1[:, 7] = 127.0                              # no plane match for holes
    for b in range(32):
        sel = np.nonzero(t_block[ega] == b)[0]
        assert len(sel) == E // 32
        res = pp[sel] % 128
        q = _assign_bins(res, ROTS, G1FIX)
        # slot within block: (q, g) with g = rank within bin q
        order = np.argsort(q, kind='stable')
        sel, q = sel[order], q[order]
        cnt = np.bincount(q, minlength=128)
        starts = np.concatenate([[0], np.cumsum(cnt)])[:-1]
        g = np.arange(len(sel)) - starts[q]
        slot = (b * G1FIX + g) * 128 + q
        xidx1[slot] = (eg[sel] >> 1).astype(np.int16)
        tab1[slot, 0] = basepts[ega[sel], 0]
        tab1[slot, 1] = basepts[ega[sel], 1]
        tab1[slot, 2] = base_weight[pp[sel], 0]
        tab1[slot, 3] = base_weight[pp[sel], 1]
        tab1[slot, 4] = (1 - (eg[sel] & 1)).astype(np.float16)
        tab1[slot, 5] = (eg[sel] & 1).astype(np.float16)
        tab1[slot, 6] = t_local[ega[sel]].astype(np.float16)
        rot_used = (q - pp[sel]) % 128
        cidx = np.zeros(len(sel), np.int64)
        for ci, r in enumerate(ROTS):
            cidx[rot_used == r] = ci
        tab1[slot, 7] = (cidx * 32 + (pp[sel] >> 7)).astype(np.float16)

    # tab1 device layout: [128, 8, SG1] (plane-major per partition)
    tab1_dev = np.ascontiguousarray(
        tab1.reshape(SG1, 128, 8).transpose(1, 2, 0)).astype(np.float16)

    # rotation matrices for rnorm planes (f16): R[q, q'] = [q' == (q+rot)%128]
    rotm = np.zeros((2, 128, 128), np.float16)
    for ci, r in enumerate(ROTS[1:]):
        rotm[ci, np.arange(128), (np.arange(128) + r) % 128] = 1.0

    # ---------------- sigma2: balanced pair blocks ----------------
    m2 = eg >> 1
    pdeg = np.bincount(m2, minlength=N // 2)
    blocks2 = _balance_blocks(pdeg, 256, 128, E // 256)
    p_local = np.empty(N // 2, np.int64)
    p_block = np.empty(N // 2, np.int64)
    p_newrow = np.empty(N // 2, np.int64)
    for b in range(256):
        p_local[blocks2[b]] = np.arange(128)
        p_block[blocks2[b]] = b
        p_newrow[blocks2[b]] = 128 * b + np.arange(128)

    SG2 = 1024
    S2 = SG2 * 128
    yidx2 = np.zeros(S2, np.int16)
    tab2 = np.zeros((S2, 8), np.float16)            # gx gy bpx bpy bwx bwy gme gmo... see below
    tab2[:, 7] = -1.0                               # prel hole marker unused (masks=0)
    slot2_of = np.empty(E, np.int64)
    pos = 0
    for b in range(256):
        sel = np.nonzero(p_block[m2] == b)[0]
        assert len(sel) == E // 256
        n = len(sel)
        slot = pos + np.arange(n)
        pos += n
        yidx2[slot] = t_newrow[ega[sel]].astype(np.int16)
        tab2[slot, 0] = grid_b[eg[sel], 0]
        tab2[slot, 1] = grid_b[eg[sel], 1]
        tab2[slot, 2] = basepts[ega[sel], 0]
        tab2[slot, 3] = basepts[ega[sel], 1]
        tab2[slot, 4] = base_weight[pp[sel], 0]
        tab2[slot, 5] = base_weight[pp[sel], 1]
        # fold parity into masks; 6 = even mask, 7 = prel; odd mask = 1-even... we
        # need BOTH me and mo plus prel: pack me in plane 6 and prel+256*mo?? no:
        # use plane 6 = prel for even-half, plane 7 = prel for odd-half: the two
        # V2 halves use separate one-hots? cheaper: keep me/mo implicit:
        # prel_even = prel if even else -1; prel_odd = prel if odd else -1.
        prel = p_local[m2[sel]].astype(np.float16)
        odd = (eg[sel] & 1).astype(bool)
        pe = prel.copy(); pe[odd] = -1.0
        po = prel.copy(); po[~odd] = -1.0
        tab2[slot, 6] = pe
        tab2[slot, 7] = po
        slot2_of[sel] = slot
    tab2_dev = np.ascontiguousarray(
        tab2.reshape(SG2, 128, 8).transpose(1, 2, 0)).astype(np.float16)

    # host finish: orig pair row = out_tbl[p_newrow[pair]]

    # dtt rows permuted by target new-row
    t_origin = np.empty(NUM_PTS, np.int64)
    t_origin[t_newrow] = np.arange(NUM_PTS)
    dtt = np.ascontiguousarray(D.T[t_origin].astype(np.float32))

    iota_row = np.tile(np.arange(128, dtype=np.float16)[None, :], (128, 1))

    return dict(
        xcat=xcat,
        dtab=dtab,
        bwd=np.ascontiguousarray(bwd.astype(np.float32)),
        wfl=np.ascontiguousarray(weights.reshape(CIN, COUT * KM).astype(np.float32)),
        dt_t=dtt,
        xidx1=_wrap16(xidx1),
        tab1=tab1_dev.reshape(128, 8 * SG1),
        rotm=np.ascontiguousarray(rotm.reshape(2 * 128, 128)),
        yidx2=_wrap16(yidx2),
        tab2=tab2_dev.reshape(128, 8 * SG2),
        iota=np.ascontiguousarray(iota_row),
    ), p_newrow


def build(nc, cfg):
    N, NUM_PTS, K = cfg["N"], cfg["NUM_PTS"], cfg["K"]
    CIN, COUT, KM = cfg["CIN"], cfg["COUT"], cfg["KM"]
    G1FIX = cfg["G1FIX"]
    S2CHUNK = cfg["S2CHUNK"]
    E = K * NUM_PTS
    PCOLS = NUM_PTS // 128
    OJ = COUT * KM
    SG1 = 32 * G1FIX
    SG2 = 1024
    NPLANES = 96

    xcat_d = nc.dram_tensor("xcat", [N // 2, 128], f16, kind="ExternalInput")
    dtab_d = nc.dram_tensor("dtab", [128, E // 128, 5], f16, kind="ExternalInput")
    bwd_d = nc.dram_tensor("bwd", [128, PCOLS, 2], f32, kind="ExternalInput")
    wfl_d = nc.dram_tensor("wfl", [CIN, OJ], f32, kind="ExternalInput")
    dtt_d = nc.dram_tensor("dt_t", [NUM_PTS, KM], f32, kind="ExternalInput")
    xidx1_d = nc.dram_tensor("xidx1", [128, SG1 * 128 // 16], i16, kind="ExternalInput")
    tab1_d = nc.dram_tensor("tab1", [128, 8 * SG1], f16, kind="ExternalInput")
    rotm_d = nc.dram_tensor("rotm", [2 * 128, 128], f16, kind="ExternalInput")
    yidx2_d = nc.dram_tensor("yidx2", [128, SG2 * 128 // 16], i16, kind="ExternalInput")
    tab2_d = nc.dram_tensor("tab2", [128, 8 * SG2], f16, kind="ExternalInput")
    iota_d = nc.dram_tensor("iota", [128, 128], f16, kind="ExternalInput")
    out_d = nc.dram_tensor("out", [N // 2, 64], f32, kind="ExternalOutput")
    ycat_d = nc.dram_tensor("ycat_tbl", [NUM_PTS, 64], f32, kind="Internal")

    with tile.TileContext(nc) as tc:
        with tc.tile_pool(name="consts", bufs=1) as cp:
            ident = cp.tile([128, 128], f32)
            masks.make_identity(nc, ident[:])
            nc.gpsimd.load_library(library_config.mlp)

            wfl = cp.tile([CIN, OJ], f32)
            nc.sync.dma_start(wfl[:], wfl_d[:])
            bwd = cp.tile([128, PCOLS * 2], f32)
            bwd3 = bwd[:].rearrange("p (q t) -> p q t", t=2)
            nc.sync.dma_start(bwd3, bwd_d[:])
            iota = cp.tile([128, 128], f16)
            nc.sync.dma_start(iota[:], iota_d[:])
            rotm = cp.tile([128, 2 * 128], f16)
            nc.sync.dma_start(rotm[:].rearrange("p (c j) -> p c j", c=2),
                              rotm_d.ap().rearrange("(c p) j -> p c j", p=128))
            tab1 = cp.tile([128, 8 * SG1], f16)
            nc.sync.dma_start(tab1[:], tab1_d[:])
            tab13 = tab1[:].rearrange("p (t s) -> p t s", t=8)
            xi1 = cp.tile([128, SG1 * 8], i16)
            nc.sync.dma_start(xi1[:], xidx1_d[:])
            tab2 = cp.tile([128, 8 * SG2], f16)
            nc.sync.dma_start(tab2[:], tab2_d[:])
            tab23 = tab2[:].rearrange("p (t s) -> p t s", t=8)
            yi2 = cp.tile([128, SG2 * 8], i16)
            nc.sync.dma_start(yi2[:], yidx2_d[:])

            # ---------- dense pass: rnorm planes ----------
            rnt = cp.tile([128, NPLANES], f16)
            with tc.tile_pool(name="dense", bufs=1) as dp, \
                    tc.tile_pool(name="dpsum", bufs=1, space="PSUM") as dq:
                JD = E // 128
                dtab = dp.tile([128, JD * 5], f16)
                dt3 = dtab[:].rearrange("p (j t) -> p j t", t=5)
                nc.sync.dma_start(dt3, dtab_d[:])
                dd0 = dp.tile([128, JD], f32)
                dd1 = dp.tile([128, JD], f32)
                nc.vector.tensor_tensor(dd0[:], dt3[:, :, 0], dt3[:, :, 3], op=subtract)
                nc.vector.tensor_tensor(dd0[:], dd0[:], dd0[:], op=mult)
                nc.vector.tensor_tensor(dd1[:], dt3[:, :, 1], dt3[:, :, 4], op=subtract)
                nc.vector.tensor_tensor(dd1[:], dd1[:], dd1[:], op=mult)
                d0k = dd0[:].rearrange("p (q k) -> p q k", k=K)
                d1k = dd1[:].rearrange("p (q k) -> p q k", k=K)
                nc.vector.tensor_tensor(d0k, d0k,
                                        bwd3[:, :, 0].broadcast_to((128, PCOLS, K)),
                                        op=mult)
                nc.vector.tensor_tensor(d1k, d1k,
                                        bwd3[:, :, 1].broadcast_to((128, PCOLS, K)),
                                        op=mult)
                nc.vector.tensor_tensor(dd0[:], dd0[:], dd1[:], op=add)
                du = dp.tile([128, JD], f32)
                nc.scalar.activation(du[:], dd0[:], Exp, scale=-1.0)
                nc.vector.tensor_tensor(du[:], du[:], dt3[:, :, 2], op=mult)
                nc.vector.tensor_tensor(du[:], du[:], du[:], op=mult)
                nsq = dp.tile([128, PCOLS], f32)
                nc.vector.reduce_sum(nsq[:].unsqueeze(2),
                                     du[:].rearrange("p (q k) -> p q k", k=K),
                                     axis=X)
                nc.scalar.activation(nsq[:], nsq[:],
                                     mybir.ActivationFunctionType.Sqrt)
                nc.vector.tensor_scalar_add(nsq[:], nsq[:], 1e-5)
                nc.vector.reciprocal(nsq[:], nsq[:])
                nc.vector.tensor_copy(rnt[:, 0:32], nsq[:])
                for ci in range(2):
                    rp = dq.tile([128, 32], f32, tag="rp")
                    nc.tensor.matmul(rp[:], rotm[:, ci * 128:(ci + 1) * 128],
                                     rnt[:, 0:32], start=True, stop=True)
                    nc.vector.tensor_copy(rnt[:, 32 + 32 * ci:64 + 32 * ci], rp[:])

            # ---------- rn_all: 96-plane select ----------
            rn_all = cp.tile([128, SG1], f16)
            with tc.tile_pool(name="rnsel", bufs=1) as rp:
                tmp = rp.tile([128, SG1], f16)
                nc.vector.memset(rn_all[:], 0.0)
                prow = tab13[:, 7, :]
                for j in range(NPLANES):
                    nc.vector.scalar_tensor_tensor(
                        tmp[:], prow, float(j),
                        rnt[:, j:j + 1].broadcast_to((128, SG1)),
                        op0=is_equal, op1=mult)
                    nc.vector.tensor_tensor(rn_all[:], rn_all[:], tmp[:], op=add)

            # ---------- sigma1 ----------
            xhT = cp.tile([CIN, NUM_PTS], f32)
            with tc.tile_pool(name="ph1", bufs=3) as p1, \
                    tc.tile_pool(name="ps1", bufs=2, space="PSUM") as q1:
                for b in range(32):
                    sl = slice(b * G1FIX, (b + 1) * G1FIX)
                    gx = p1.tile([128, G1FIX * 128], f16, tag="gx")
                    gx3 = gx[:].rearrange("p (g e) -> p g e", e=128)
                    nc.gpsimd.dma_gather(
                        gx3, xcat_d[:],
                        xi1[:, b * G1FIX * 8:(b + 1) * G1FIX * 8],
                        G1FIX * 128, G1FIX * 128, 128,
                        elem_step=128, single_packet=False)
                    me = tab13[:, 4, sl]
                    mo = tab13[:, 5, sl]
                    # grid/gw of the edge's node via parity select
                    ge = p1.tile([128, G1FIX * 3], f32, tag="ge")
                    ge3 = ge[:].rearrange("p (g t) -> p g t", t=3)
                    t0 = p1.tile([128, G1FIX * 3], f32, tag="t0")
                    t03 = t0[:].rearrange("p (g t) -> p g t", t=3)
                    nc.vector.tensor_tensor(
                        ge3, gx3[:, :, 32:35],
                        me.unsqueeze(2).broadcast_to((128, G1FIX, 3)), op=mult)
                    nc.vector.tensor_tensor(
                        t03, gx3[:, :, 96:99],
                        mo.unsqueeze(2).broadcast_to((128, G1FIX, 3)), op=mult)
                    nc.vector.tensor_tensor(ge3, ge3, t03, op=add)
                    dd = p1.tile([128, G1FIX * 2], f32, tag="dd")
                    dd3 = dd[:].rearrange("p (g t) -> p g t", t=2)
                    nc.vector.tensor_tensor(
                        dd3, ge3[:, :, 0:2],
                        tab13[:, 0:2, sl].rearrange("p t s -> p s t"), op=subtract)
                    nc.vector.tensor_tensor(dd3, dd3, dd3, op=mult)
                    nc.vector.tensor_tensor(
                        dd3, dd3,
                        tab13[:, 2:4, sl].rearrange("p t s -> p s t"), op=mult)
                    ga = p1.tile([128, G1FIX], f32, tag="ga")
                    nc.vector.tensor_tensor(ga[:], dd3[:, :, 0], dd3[:, :, 1],
                                            op=add)
                    nc.scalar.activation(ga[:], ga[:], Exp, scale=-1.0)
                    nc.vector.tensor_tensor(ga[:], ga[:], ge3[:, :, 2], op=mult)
                    nc.vector.tensor_tensor(ga[:], ga[:], rn_all[:, sl], op=mult)
                    wlo = p1.tile([128, G1FIX], f32, tag="wlo")
                    whi = p1.tile([128, G1FIX], f32, tag="whi")
                    nc.vector.tensor_tensor(wlo[:], ga[:], me, op=mult)
                    nc.vector.tensor_tensor(whi[:], ga[:], mo, op=mult)
                    v1 = p1.tile([128, G1FIX * CIN], f16, tag="v1")
                    v13 = v1[:].rearrange("p (g e) -> p g e", e=CIN)
                    t1 = p1.tile([128, G1FIX * CIN], f16, tag="t1")
                    t13 = t1[:].rearrange("p (g e) -> p g e", e=CIN)
                    nc.vector.tensor_tensor(
                        v13, gx3[:, :, 0:CIN],
                        wlo[:].unsqueeze(2).broadcast_to((128, G1FIX, CIN)),
                        op=mult)
                    nc.vector.tensor_tensor(
                        t13, gx3[:, :, 64:64 + CIN],
                        whi[:].unsqueeze(2).broadcast_to((128, G1FIX, CIN)),
                        op=mult)
                    nc.vector.tensor_tensor(v13, v13, t13, op=add)
                    oh = p1.tile([128, G1FIX * 128], f16, tag="oh")
                    oh3 = oh[:].rearrange("p (g e) -> p g e", e=128)
                    nc.vector.tensor_tensor(
                        oh3,
                        tab13[:, 6, sl].unsqueeze(2).broadcast_to((128, G1FIX, 128)),
                        iota[:].unsqueeze(1).broadcast_to((128, G1FIX, 128)),
                        op=is_equal)
                    ps = q1.tile([CIN, 128], f32, tag="pxh")
                    for g in range(G1FIX):
                        nc.tensor.matmul(ps[:], v13[:, g, :], oh3[:, g, :],
                                         start=(g == 0), stop=(g == G1FIX - 1))
                    nc.vector.tensor_copy(xhT[:, b * 128:(b + 1) * 128], ps[:])

            # ---------- phase C ----------
            with tc.tile_pool(name="phc", bufs=2) as pc, \
                    tc.tile_pool(name="psc", bufs=2, space="PSUM") as qc:
                for t in range(PCOLS):
                    o1p = qc.tile([128, OJ], f32, tag="o1p")
                    nc.tensor.matmul(o1p[:], xhT[:, t * 128:(t + 1) * 128],
                                     wfl[:], start=True, stop=True)
                    dtt = pc.tile([128, KM], f32, tag="dtt")
                    nc.sync.dma_start(dtt[:], dtt_d[t * 128:(t + 1) * 128, :])
                    o1 = pc.tile([128, OJ], f32, tag="o1")
                    nc.vector.tensor_tensor(
                        o1[:].rearrange("p (o j) -> p o j", j=KM),
                        o1p[:].rearrange("p (o j) -> p o j", j=KM),
                        dtt[:].unsqueeze(1).broadcast_to((128, COUT, KM)),
                        op=mult)
                    yrow = pc.tile([128, 64], f32, tag="yrow")
                    nc.vector.reduce_sum(
                        yrow[:, 0:COUT].unsqueeze(2),
                        o1[:].rearrange("p (o j) -> p o j", j=KM), axis=X)
                    nc.sync.dma_start(
                        ycat_d.ap()[t * 128:(t + 1) * 128, 0:COUT], yrow[:, 0:COUT])

            # ---------- sigma2 ----------
            NCH = 256 // S2CHUNK          # chunks
            GC = S2CHUNK * 4              # group-columns per chunk (G2FIX=4)
            with tc.tile_pool(name="ph2", bufs=2) as p2, \
                    tc.tile_pool(name="ps2", bufs=2, space="PSUM") as q2:
                for c in range(NCH):
                    s0 = c * GC           # first group-col of chunk
                    sl = slice(s0, s0 + GC)
                    gy = p2.tile([128, GC * 64], f32, tag="gy")
                    gy3 = gy[:].rearrange("p (g e) -> p g e", e=64)
                    nc.gpsimd.dma_gather(
                        gy3, ycat_d[:],
                        yi2[:, s0 * 8:(s0 + GC) * 8],
                        GC * 128, GC * 128, 64,
                        elem_step=64, single_packet=False)
                    dd = p2.tile([128, GC * 2], f32, tag="dd2")
                    dd3 = dd[:].rearrange("p (g t) -> p g t", t=2)
                    nc.vector.tensor_tensor(
                        dd3, tab23[:, 0:2, sl].rearrange("p t s -> p s t"),
                        tab23[:, 2:4, sl].rearrange("p t s -> p s t"), op=subtract)
                    nc.vector.tensor_tensor(dd3, dd3, dd3, op=mult)
                    nc.vector.tensor_tensor(
                        dd3, dd3,
                        tab23[:, 4:6, sl].rearrange("p t s -> p s t"), op=mult)
                    ga = p2.tile([128, GC], f32, tag="ga2")
                    nc.vector.tensor_tensor(ga[:], dd3[:, :, 0], dd3[:, :, 1],
                                            op=add)
                    nc.scalar.activation(ga[:], ga[:], Exp, scale=-1.0)
                    v2 = p2.tile([128, GC * 32], f16, tag="v2")
                    v23 = v2[:].rearrange("p (g e) -> p g e", e=32)
                    nc.vector.tensor_tensor(
                        v23, gy3[:, :, 0:32],
                        ga[:].unsqueeze(2).broadcast_to((128, GC, 32)), op=mult)
                    # one-hots: separate planes for even/odd target halves
                    ohe = p2.tile([128, GC * 128], f16, tag="ohe")
                    ohe3 = ohe[:].rearrange("p (g e) -> p g e", e=128)
                    nc.vector.tensor_tensor(
                        ohe3,
                        tab23[:, 6, sl].unsqueeze(2).broadcast_to((128, GC, 128)),
                        iota[:].unsqueeze(1).broadcast_to((128, GC, 128)),
                        op=is_equal)
                    oho = p2.tile([128, GC * 128], f16, tag="oho")
                    oho3 = oho[:].rearrange("p (g e) -> p g e", e=128)
                    nc.vector.tensor_tensor(
                        oho3,
                        tab23[:, 7, sl].unsqueeze(2).broadcast_to((128, GC, 128)),
                        iota[:].unsqueeze(1).broadcast_to((128, GC, 128)),
                        op=is_equal)
                    ob = p2.tile([128, S2CHUNK * 64], f32, tag="ob")
                    ob3 = ob[:].rearrange("p (k e) -> p k e", e=64)
                    for k in range(S2CHUNK):
                        po = q2.tile([128, 64], f32, tag="po")
                        po3 = po[:].rearrange("p (h e) -> p h e", e=32)
                        for g in range(4):
                            gc = 4 * k + g
                            nc.tensor.matmul(po3[:, 0, :], ohe3[:, gc, :],
                                             v23[:, gc, :],
                                             start=(g == 0), stop=(g == 3))
                        for g in range(4):
                            gc = 4 * k + g
                            nc.tensor.matmul(po3[:, 1, :], oho3[:, gc, :],
                                             v23[:, gc, :],
                                             start=(g == 0), stop=(g == 3))
                        nc.vector.tensor_copy(ob3[:, k, :], po[:])
                    nc.sync.dma_start(
                        out_d.ap()[c * S2CHUNK * 128:(c + 1) * S2CHUNK * 128, :]
                        .rearrange("(k p) e -> p k e", p=128),
                        ob3)
    return nc


def make_in_maps(cfg, x, grid, grid_weight, edge_grid, edge_Gauss, basepts,
                 base_weight, D, weights):
    maps, invs = [], []
    for b in range(x.shape[0]):
        m, inv2 = host_prep(cfg, x[b], grid[b], grid_weight[b], edge_grid[b],
                            edge_Gauss[b], basepts, base_weight, D, weights)
        maps.append(m)
        invs.append(inv2)
    return maps, invs


def finish(cfg, out_tbl, p_newrow):
    # device row p_newrow[pair] holds pair's output
    o = out_tbl[p_newrow]
    return np.ascontiguousarray(
        o.reshape(cfg["N"], 32)[:, :cfg["COUT"]].T)


_BUILT = {}


def _get_nc():
    if "nc" not in _BUILT:
        nc = bacc.Bacc("TRN2", target_bir_lowering=False,
                       dynamic_dma_scratch_size=32768)
        build(nc, CFG)
        nc.compile()
        _BUILT["nc"] = nc
    return _BUILT["nc"]


def kernel(x, grid, grid_weight, edge_grid, edge_Gauss, basepts, base_weight,
           D, weights, _trace=False):
    cfg = CFG
    in_maps, invs = make_in_maps(
        cfg, np.asarray(x, np.float32), np.asarray(grid),
        np.asarray(grid_weight), np.asarray(edge_grid),
        np.asarray(edge_Gauss), np.asarray(basepts),
        np.asarray(base_weight), np.asarray(D), np.asarray(weights))
    nc = _get_nc()
    res = bass_utils.run_bass_kernel_spmd(
        nc, in_maps, core_ids=list(range(x.shape[0])), trace=_trace)
    out = np.stack([finish(cfg, res.results[b]["out"], invs[b])
                    for b in range(x.shape[0])])
    kernel.last_result = res
    return out


# revision 16
# speedup vs baseline: 2.3970x; 1.1500x over previous
"""GPDconv (GNN message passing) Trainium2 Bass kernel — PE one-hot design.

Batch-parallel over 8 NeuronCores (one batch per core). The previous design
spent ~4ms/core in Q7 SWDGE descriptor generation (~8ns per gather index,
~500k indices). This version keeps exactly TWO per-edge SWDGE passes (the
provable floor) and does all aggregation on the PE via one-hot matmuls:

  sigma1: edges sorted into 32 host-balanced target-blocks (128 ega-targets,
    exactly 4096 edges each). One dma_gather of x pair-rows per edge
    (+ ~6% slot padding from the rnorm partition constraint). Per 128-edge
    group: V1 = u*rnorm*x_row, one-hot over within-block target -> PE matmul
    accumulating x_hat^T [32ch, 128t] in PSUM. rnorm[p] is delivered by a
    96-plane select: edge partition q == (p + rot_c) % 128 for one of three
    rotations (3-choice load balancing), rnorm planes live at [q, 32c+j].
  C: y = (x_hat @ W) . D^T per 128-target tile (targets in permuted order).
  sigma2: edges sorted into 256 host-balanced pair-blocks (128 node-pairs,
    exactly 512 edges each). One dma_gather of y rows per edge (zero pad).
    V2 = gauss*(parity masks)*y, one-hot over within-block pair -> PE matmul
    -> out pair-rows [128, 64] per block, written permuted; host unpermutes.

Host does index/layout prep only (sorting, balancing, packing, int16);
all value math (gauss, norms, products, reductions) runs on device.
"""
import sys

if '/opt/trn_rl_repo' not in sys.path:
    sys.path.insert(0, '/opt/trn_rl_repo')

import numpy as np
import concourse.bacc as bacc
import concourse.mybir as mybir
import concourse.tile as tile
from concourse import bass_utils, library_config, masks

f32 = mybir.dt.float32
f16 = mybir.dt.float16
i16 = mybir.dt.int16

CFG = dict(N=65536, NUM_PTS=4096, K=32, CIN=32, COUT=32, KM=16,
           G1FIX=34, ROTS=(0, 43, 86), S2CHUNK=8)

mult, add, subtract = (mybir.AluOpType.mult, mybir.AluOpType.add,
                       mybir.AluOpType.subtract)
is_equal = mybir.AluOpType.is_equal
Exp = mybir.ActivationFunctionType.Exp
X = mybir.AxisListType.X


def _wrap16(a):
    return np.ascontiguousarray(np.tile(a.reshape(-1, 16).T, (8, 1)))


def _balance_blocks(deg, nblocks, per_block_items, per_block_sum):
    """Partition items into nblocks of exactly per_block_items items with
    degree sums exactly per_block_sum. Snake-deal + exact swap repair."""
    deg = np.asarray(deg, np.int64)
    n = len(deg)
    assert n == nblocks * per_block_items
    assert deg.sum() == nblocks * per_block_sum
    order = np.argsort(-deg, kind='stable')
    # snake deal: rows of nblocks, alternating direction
    rows = order.reshape(per_block_items, nblocks)
    for r in range(1, per_block_items, 2):
        rows[r] = rows[r][::-1]
    blocks = [list(rows[:, b]) for b in range(nblocks)]
    sums = np.array([deg[b].sum() for b in blocks], np.int64)
    for _ in range(100000):
        dev = sums - per_block_sum
        if not dev.any():
            break
        hi = int(np.argmax(dev))
        lo = int(np.argmin(dev))
        dstar = int(min(dev[hi], -dev[lo]))
        ha = np.asarray(blocks[hi])
        la = np.asarray(blocks[lo])
        da, db = deg[ha], deg[la]
        ua = np.unique(da)
        ub = np.unique(db)
        found = None
        for want in range(dstar, 0, -1):
            hit = ua[np.isin(ua - want, ub)]
            if len(hit):
                va = int(hit[0])
                ai = int(np.nonzero(da == va)[0][0])
                bj = int(np.nonzero(db == va - want)[0][0])
                found = (ai, bj, want)
                break
        assert found is not None, (dev[hi], dev[lo], ua, ub)
        ai, bj, want = found
        a_it, b_it = int(ha[ai]), int(la[bj])
        blocks[hi][ai] = b_it
        blocks[lo][bj] = a_it
        sums[hi] -= want
        sums[lo] += want
    assert (sums == per_block_sum).all(), sums
    return [np.asarray(b, np.int64) for b in blocks]


def _assign_bins(res, rots, cap):
    """3-choice capacitated assignment: edge i may go to bin
    (res[i]+rot)%128; return bin per edge with loads <= cap.
    Greedy lightest-bin init + BFS augmenting-path eviction."""
    n = len(res)
    nr = len(rots)
    cands = np.stack([(res + r) % 128 for r in rots], 1)   # (n, nr)
    cnt = np.zeros(128, np.int64)
    choice = np.zeros(n, np.int64)
    order = np.random.default_rng(0).permutation(n)
    for i in order:
        c = cands[i]
        j = int(np.argmin(cnt[c]))
        choice[i] = j
        cnt[c[j]] += 1
    # bin -> member edge list
    members = [[] for _ in range(128)]
    for i in range(n):
        members[int(cands[i, choice[i]])].append(i)
    while True:
        over = [b for b in range(128) if cnt[b] > cap]
        if not over:
            break
        s = over[0]
        # BFS from s to any bin with load < cap via edge reassignments
        parent = {s: None}
        frontier = [s]
        goal = None
        while frontier and goal is None:
            nxt = []
            for u in frontier:
                for i in members[u]:
                    for j in range(nr):
                        v = int(cands[i, j])
                        if v == u or v in parent:
                            continue
                        parent[v] = (u, i, j)
                        if cnt[v] < cap:
                            goal = v
                            break
                        nxt.append(v)
                    if goal is not None:
                        break
                if goal is not None:
                    break
            frontier = nxt
        assert goal is not None, "no augmenting path; raise G1FIX"
        # walk back, reassigning one edge per hop
        v = goal
        while parent[v] is not None:
            u, i, j = parent[v]
            members[u].remove(i)
            members[v].append(i)
            choice[i] = j
            cnt[u] -= 1
            cnt[v] += 1
            v = u
    assert cnt.max() <= cap, (cnt.max(), cap)
    return cands[np.arange(n), choice]


def host_prep(cfg, x_b, grid_b, gw_b, eg_b, ega_b, basepts, base_weight, D,
              weights):
    N, NUM_PTS, K = cfg["N"], cfg["NUM_PTS"], cfg["K"]
    CIN, COUT, KM = cfg["CIN"], cfg["COUT"], cfg["KM"]
    G1FIX, ROTS = cfg["G1FIX"], cfg["ROTS"]
    E = K * NUM_PTS
    PCOLS = NUM_PTS // 128
    eg = eg_b.T.reshape(-1).astype(np.int64)        # (E,) [k, p] order
    ega = ega_b.T.reshape(-1).astype(np.int64)
    pp = np.tile(np.arange(NUM_PTS), K)

    # ---------------- xcat pair-row table ----------------
    rows = np.zeros((N, 64), np.float32)
    rows[:, :CIN] = x_b.T
    rows[:, CIN] = grid_b[:, 0]
    rows[:, CIN + 1] = grid_b[:, 1]
    rows[:, CIN + 2] = gw_b
    xcat = rows.astype(np.float16).reshape(N // 2, 128)

    # ---------------- dense tab (rnorm pass) ----------------
    def lay_dense(v):
        return np.ascontiguousarray(
            v.reshape(K, PCOLS, 128).transpose(2, 1, 0).reshape(128, E // 128))
    dtab = np.stack([
        lay_dense(grid_b[eg, 0].reshape(K, NUM_PTS)),
        lay_dense(grid_b[eg, 1].reshape(K, NUM_PTS)),
        lay_dense(gw_b[eg].reshape(K, NUM_PTS)),
        lay_dense(basepts[ega, 0].reshape(K, NUM_PTS)),
        lay_dense(basepts[ega, 1].reshape(K, NUM_PTS)),
    ], axis=-1).astype(np.float16)
    bwd = np.stack([base_weight[:, 0].reshape(PCOLS, 128).T,
                    base_weight[:, 1].reshape(PCOLS, 128).T], axis=-1)

    # ---------------- sigma1: balanced target blocks ----------------
    tdeg = np.bincount(ega, minlength=NUM_PTS)
    blocks1 = _balance_blocks(tdeg, 32, 128, E // 32)
    t_newrow = np.empty(NUM_PTS, np.int64)          # orig target -> new row
    t_local = np.empty(NUM_PTS, np.int64)
    t_block = np.empty(NUM_PTS, np.int64)
    for b in range(32):
        t_newrow[blocks1[b]] = 128 * b + np.arange(128)
        t_local[blocks1[b]] = np.arange(128)
        t_block[blocks1[b]] = b

    SG1 = 32 * G1FIX
    S1 = SG1 * 128
    xidx1 = np.zeros(S1, np.int16)
    tab1 = np.zeros((S1, 8), np.float16)            # bpx bpy bwx bwy me mo egar prow
    tab1[:, 6] = -1.0
    tab1[:, 7] = 127.0                              # no plane match for holes
    for b in range(32):
        sel = np.nonzero(t_block[ega] == b)[0]
        assert len(sel) == E // 32
        res = pp[sel] % 128
        q = _assign_bins(res, ROTS, G1FIX)
        # slot within block: (q, g) with g = rank within bin q
        order = np.argsort(q, kind='stable')
        sel, q = sel[order], q[order]
        cnt = np.bincount(q, minlength=128)
        starts = np.concatenate([[0], np.cumsum(cnt)])[:-1]
        g = np.arange(len(sel)) - starts[q]
        slot = (b * G1FIX + g) * 128 + q
        xidx1[slot] = (eg[sel] >> 1).astype(np.int16)
        tab1[slot, 0] = basepts[ega[sel], 0]
        tab1[slot, 1] = basepts[ega[sel], 1]
        tab1[slot, 2] = base_weight[pp[sel], 0]
        tab1[slot, 3] = base_weight[pp[sel], 1]
        tab1[slot, 4] = (1 - (eg[sel] & 1)).astype(np.float16)
        tab1[slot, 5] = (eg[sel] & 1).astype(np.float16)
        tab1[slot, 6] = t_local[ega[sel]].astype(np.float16)
        rot_used = (q - pp[sel]) % 128
        cidx = np.zeros(len(sel), np.int64)
        for ci, r in enumerate(ROTS):
            cidx[rot_used == r] = ci
        tab1[slot, 7] = (cidx * 32 + (pp[sel] >> 7)).astype(np.float16)

    # tab1 device layout: [128, 8, SG1] (plane-major per partition)
    tab1_dev = np.ascontiguousarray(
        tab1.reshape(SG1, 128, 8).transpose(1, 2, 0)).astype(np.float16)

    # rotation matrices for rnorm planes (f16): R[q, q'] = [q' == (q+rot)%128]
    rotm = np.zeros((2, 128, 128), np.float16)
    for ci, r in enumerate(ROTS[1:]):
        rotm[ci, np.arange(128), (np.arange(128) + r) % 128] = 1.0

    # ---------------- sigma2: balanced pair blocks ----------------
    m2 = eg >> 1
    pdeg = np.bincount(m2, minlength=N // 2)
    blocks2 = _balance_blocks(pdeg, 256, 128, E // 256)
    p_local = np.empty(N // 2, np.int64)
    p_block = np.empty(N // 2, np.int64)
    p_newrow = np.empty(N // 2, np.int64)
    for b in range(256):
        p_local[blocks2[b]] = np.arange(128)
        p_block[blocks2[b]] = b
        p_newrow[blocks2[b]] = 128 * b + np.arange(128)

    SG2 = 1024
    S2 = SG2 * 128
    yidx2 = np.zeros(S2, np.int16)
    tab2 = np.zeros((S2, 8), np.float16)            # gx gy bpx bpy bwx bwy gme gmo... see below
    tab2[:, 7] = -1.0                               # prel hole marker unused (masks=0)
    slot2_of = np.empty(E, np.int64)
    pos = 0
    for b in range(256):
        sel = np.nonzero(p_block[m2] == b)[0]
        assert len(sel) == E // 256
        n = len(sel)
        slot = pos + np.arange(n)
        pos += n
        yidx2[slot] = t_newrow[ega[sel]].astype(np.int16)
        tab2[slot, 0] = grid_b[eg[sel], 0]
        tab2[slot, 1] = grid_b[eg[sel], 1]
        tab2[slot, 2] = basepts[ega[sel], 0]
        tab2[slot, 3] = basepts[ega[sel], 1]
        tab2[slot, 4] = base_weight[pp[sel], 0]
        tab2[slot, 5] = base_weight[pp[sel], 1]
        # fold parity into masks; 6 = even mask, 7 = prel; odd mask = 1-even... we
        # need BOTH me and mo plus prel: pack me in plane 6 and prel+256*mo?? no:
        # use plane 6 = prel for even-half, plane 7 = prel for odd-half: the two
        # V2 halves use separate one-hots? cheaper: keep me/mo implicit:
        # prel_even = prel if even else -1; prel_odd = prel if odd else -1.
        prel = p_local[m2[sel]].astype(np.float16)
        odd = (eg[sel] & 1).astype(bool)
        pe = prel.copy(); pe[odd] = -1.0
        po = prel.copy(); po[~odd] = -1.0
        tab2[slot, 6] = pe
        tab2[slot, 7] = po
        slot2_of[sel] = slot
    tab2_dev = np.ascontiguousarray(
        tab2.reshape(SG2, 128, 8).transpose(1, 2, 0)).astype(np.float16)

    # host finish: orig pair row = out_tbl[p_newrow[pair]]

    # dtt rows permuted by target new-row
    t_origin = np.empty(NUM_PTS, np.int64)
    t_origin[t_newrow] = np.arange(NUM_PTS)
    dtt = np.ascontiguousarray(D.T[t_origin].astype(np.float32))

    iota_row = np.tile(np.arange(128, dtype=np.float16)[None, :], (128, 1))

    return dict(
        xcat=xcat,
        dtab=dtab,
        bwd=np.ascontiguousarray(bwd.astype(np.float32)),
        wfl=np.ascontiguousarray(weights.reshape(CIN, COUT * KM).astype(np.float32)),
        dt_t=dtt,
        xidx1=_wrap16(xidx1),
        tab1=tab1_dev.reshape(128, 8 * SG1),
        rotm=np.ascontiguousarray(rotm.reshape(2 * 128, 128)),
        yidx2=_wrap16(yidx2),
        tab2=tab2_dev.reshape(128, 8 * SG2),
        iota=np.ascontiguousarray(iota_row),
    ), p_newrow


def build(nc, cfg):
    N, NUM_PTS, K = cfg["N"], cfg["NUM_PTS"], cfg["K"]
    CIN, COUT, KM = cfg["CIN"], cfg["COUT"], cfg["KM"]
    G1FIX = cfg["G1FIX"]
    S2CHUNK = cfg["S2CHUNK"]
    E = K * NUM_PTS
    PCOLS = NUM_PTS // 128
    OJ = COUT * KM
    SG1 = 32 * G1FIX
    SG2 = 1024
    NPLANES = 96

    xcat_d = nc.dram_tensor("xcat", [N // 2, 128], f16, kind="ExternalInput")
    dtab_d = nc.dram_tensor("dtab", [128, E // 128, 5], f16, kind="ExternalInput")
    bwd_d = nc.dram_tensor("bwd", [128, PCOLS, 2], f32, kind="ExternalInput")
    wfl_d = nc.dram_tensor("wfl", [CIN, OJ], f32, kind="ExternalInput")
    dtt_d = nc.dram_tensor("dt_t", [NUM_PTS, KM], f32, kind="ExternalInput")
    xidx1_d = nc.dram_tensor("xidx1", [128, SG1 * 128 // 16], i16, kind="ExternalInput")
    tab1_d = nc.dram_tensor("tab1", [128, 8 * SG1], f16, kind="ExternalInput")
    rotm_d = nc.dram_tensor("rotm", [2 * 128, 128], f16, kind="ExternalInput")
    yidx2_d = nc.dram_tensor("yidx2", [128, SG2 * 128 // 16], i16, kind="ExternalInput")
    tab2_d = nc.dram_tensor("tab2", [128, 8 * SG2], f16, kind="ExternalInput")
    iota_d = nc.dram_tensor("iota", [128, 128], f16, kind="ExternalInput")
    out_d = nc.dram_tensor("out", [N // 2, 64], f32, kind="ExternalOutput")
    ycat_d = nc.dram_tensor("ycat_tbl", [NUM_PTS, 64], f32, kind="Internal")

    with tile.TileContext(nc) as tc:
        with tc.tile_pool(name="consts", bufs=1) as cp:
            ident = cp.tile([128, 128], f32)
            masks.make_identity(nc, ident[:])
            nc.gpsimd.load_library(library_config.mlp)

            wfl = cp.tile([CIN, OJ], f32)
            nc.sync.dma_start(wfl[:], wfl_d[:])
            bwd = cp.tile([128, PCOLS * 2], f32)
            bwd3 = bwd[:].rearrange("p (q t) -> p q t", t=2)
            nc.sync.dma_start(bwd3, bwd_d[:])
            iota = cp.tile([128, 128], f16)
            nc.sync.dma_start(iota[:], iota_d[:])
            rotm = cp.tile([128, 2 * 128], f16)
            nc.sync.dma_start(rotm[:].rearrange("p (c j) -> p c j", c=2),
                              rotm_d.ap().rearrange("(c p) j -> p c j", p=128))
            tab1 = cp.tile([128, 8 * SG1], f16)
            nc.sync.dma_start(tab1[:], tab1_d[:])
            tab13 = tab1[:].rearrange("p (t s) -> p t s", t=8)
            xi1 = cp.tile([128, SG1 * 8], i16)
            nc.sync.dma_start(xi1[:], xidx1_d[:])


            # ---------- dense pass: rnorm planes (cp pool: no SBUF reuse
            # so sigma1 gathers are not blocked behind this) ----------
            rnt = cp.tile([128, NPLANES], f16)
            with tc.tile_pool(name="dpsum", bufs=1, space="PSUM") as dq:
                JD = E // 128
                dtab = cp.tile([128, JD * 5], f16)
                dt3 = dtab[:].rearrange("p (j t) -> p j t", t=5)
                nc.sync.dma_start(dt3, dtab_d[:])
                dd0 = cp.tile([128, JD], f32)
                dd1 = cp.tile([128, JD], f32)
                nc.vector.tensor_tensor(dd0[:], dt3[:, :, 0], dt3[:, :, 3], op=subtract)
                nc.vector.tensor_tensor(dd0[:], dd0[:], dd0[:], op=mult)
                nc.vector.tensor_tensor(dd1[:], dt3[:, :, 1], dt3[:, :, 4], op=subtract)
                nc.vector.tensor_tensor(dd1[:], dd1[:], dd1[:], op=mult)
                d0k = dd0[:].rearrange("p (q k) -> p q k", k=K)
                d1k = dd1[:].rearrange("p (q k) -> p q k", k=K)
                nc.vector.tensor_tensor(d0k, d0k,
                                        bwd3[:, :, 0].broadcast_to((128, PCOLS, K)),
                                        op=mult)
                nc.vector.tensor_tensor(d1k, d1k,
                                        bwd3[:, :, 1].broadcast_to((128, PCOLS, K)),
                                        op=mult)
                nc.vector.tensor_tensor(dd0[:], dd0[:], dd1[:], op=add)
                nc.scalar.activation(dd1[:], dd0[:], Exp, scale=-1.0)
                nc.vector.tensor_tensor(dd1[:], dd1[:], dt3[:, :, 2], op=mult)
                nc.vector.tensor_tensor(dd1[:], dd1[:], dd1[:], op=mult)
                nsq = cp.tile([128, PCOLS], f32)
                nc.vector.reduce_sum(nsq[:].unsqueeze(2),
                                     dd1[:].rearrange("p (q k) -> p q k", k=K),
                                     axis=X)
                nc.scalar.activation(nsq[:], nsq[:],
                                     mybir.ActivationFunctionType.Sqrt)
                nc.vector.tensor_scalar_add(nsq[:], nsq[:], 1e-5)
                nc.vector.reciprocal(nsq[:], nsq[:])
                nc.vector.tensor_copy(rnt[:, 0:32], nsq[:])
                for ci in range(2):
                    rp = dq.tile([128, 32], f32, tag="rp")
                    nc.tensor.matmul(rp[:], rotm[:, ci * 128:(ci + 1) * 128],
                                     rnt[:, 0:32], start=True, stop=True)
                    nc.vector.tensor_copy(rnt[:, 32 + 32 * ci:64 + 32 * ci], rp[:])

            # ---------- rn_all: 96-plane select ----------
            rn_all = cp.tile([128, SG1], f16)
            rtmp = cp.tile([128, SG1], f16)
            nc.vector.memset(rn_all[:], 0.0)
            prow = tab13[:, 7, :]
            for j in range(NPLANES):
                nc.vector.scalar_tensor_tensor(
                    rtmp[:], prow, float(j),
                    rnt[:, j:j + 1].broadcast_to((128, SG1)),
                    op0=is_equal, op1=mult)
                nc.vector.tensor_tensor(rn_all[:], rn_all[:], rtmp[:], op=add)

            # ---------- sigma1 + fused phase C ----------
            xhT = cp.tile([CIN, NUM_PTS], f32)
            with tc.tile_pool(name="ph1", bufs=3) as p1, \
                    tc.tile_pool(name="ps1", bufs=2, space="PSUM") as q1:
                for b in range(32):
                    sl = slice(b * G1FIX, (b + 1) * G1FIX)
                    gx = p1.tile([128, G1FIX * 128], f16, tag="gx", bufs=5)
                    gx3 = gx[:].rearrange("p (g e) -> p g e", e=128)
                    nc.gpsimd.dma_gather(
                        gx3, xcat_d[:],
                        xi1[:, b * G1FIX * 8:(b + 1) * G1FIX * 8],
                        G1FIX * 128, G1FIX * 128, 128,
                        elem_step=128, single_packet=False)
                    me = tab13[:, 4, sl]
                    mo = tab13[:, 5, sl]
                    # grid/gw of the edge's node via parity select
                    ge = p1.tile([128, G1FIX * 3], f32, tag="ge")
                    ge3 = ge[:].rearrange("p (g t) -> p g t", t=3)
                    t0 = p1.tile([128, G1FIX * 3], f32, tag="t0")
                    t03 = t0[:].rearrange("p (g t) -> p g t", t=3)
                    nc.vector.tensor_tensor(
                        ge3, gx3[:, :, 32:35],
                        me.unsqueeze(2).broadcast_to((128, G1FIX, 3)), op=mult)
                    nc.vector.tensor_tensor(
                        t03, gx3[:, :, 96:99],
                        mo.unsqueeze(2).broadcast_to((128, G1FIX, 3)), op=mult)
                    nc.vector.tensor_tensor(ge3, ge3, t03, op=add)
                    dd = p1.tile([128, G1FIX * 2], f32, tag="dd")
                    dd3 = dd[:].rearrange("p (g t) -> p g t", t=2)
                    nc.vector.tensor_tensor(
                        dd3, ge3[:, :, 0:2],
                        tab13[:, 0:2, sl].rearrange("p t s -> p s t"), op=subtract)
                    nc.vector.tensor_tensor(dd3, dd3, dd3, op=mult)
                    nc.vector.tensor_tensor(
                        dd3, dd3,
                        tab13[:, 2:4, sl].rearrange("p t s -> p s t"), op=mult)
                    ga = p1.tile([128, G1FIX], f32, tag="ga")
                    nc.vector.tensor_tensor(ga[:], dd3[:, :, 0], dd3[:, :, 1],
                                            op=add)
                    nc.scalar.activation(ga[:], ga[:], Exp, scale=-1.0)
                    nc.vector.tensor_tensor(ga[:], ga[:], ge3[:, :, 2], op=mult)
                    nc.vector.tensor_tensor(ga[:], ga[:], rn_all[:, sl], op=mult)
                    wlo = p1.tile([128, G1FIX], f32, tag="wlo")
                    whi = p1.tile([128, G1FIX], f32, tag="whi")
                    nc.vector.tensor_tensor(wlo[:], ga[:], me, op=mult)
                    nc.vector.tensor_tensor(whi[:], ga[:], mo, op=mult)
                    v1 = p1.tile([128, G1FIX * CIN], f16, tag="v1")
                    v13 = v1[:].rearrange("p (g e) -> p g e", e=CIN)
                    t1 = p1.tile([128, G1FIX * CIN], f16, tag="t1")
                    t13 = t1[:].rearrange("p (g e) -> p g e", e=CIN)
                    nc.vector.tensor_tensor(
                        v13, gx3[:, :, 0:CIN],
                        wlo[:].unsqueeze(2).broadcast_to((128, G1FIX, CIN)),
                        op=mult)
                    nc.vector.tensor_tensor(
                        t13, gx3[:, :, 64:64 + CIN],
                        whi[:].unsqueeze(2).broadcast_to((128, G1FIX, CIN)),
                        op=mult)
                    nc.vector.tensor_tensor(v13, v13, t13, op=add)
                    oh = p1.tile([128, G1FIX * 128], f16, tag="oh")
                    oh3 = oh[:].rearrange("p (g e) -> p g e", e=128)
                    nc.vector.tensor_tensor(
                        oh3,
                        tab13[:, 6, sl].unsqueeze(2).broadcast_to((128, G1FIX, 128)),
                        iota[:].unsqueeze(1).broadcast_to((128, G1FIX, 128)),
                        op=is_equal)
                    ps = q1.tile([CIN, 128], f32, tag="pxh")
                    for g in range(G1FIX):
                        nc.tensor.matmul(ps[:], v13[:, g, :], oh3[:, g, :],
                                         start=(g == 0), stop=(g == G1FIX - 1))
                    nc.vector.tensor_copy(xhT[:, b * 128:(b + 1) * 128], ps[:])
                    # fused phase C for this 128-target tile
                    o1p = q1.tile([128, OJ], f32, tag="o1p")
                    nc.tensor.matmul(o1p[:], xhT[:, b * 128:(b + 1) * 128],
                                     wfl[:], start=True, stop=True)
                    dtt = p1.tile([128, KM], f32, tag="dtt")
                    nc.sync.dma_start(dtt[:], dtt_d[b * 128:(b + 1) * 128, :])
                    o1 = p1.tile([128, OJ], f32, tag="o1")
                    nc.vector.tensor_tensor(
                        o1[:].rearrange("p (o j) -> p o j", j=KM),
                        o1p[:].rearrange("p (o j) -> p o j", j=KM),
                        dtt[:].unsqueeze(1).broadcast_to((128, COUT, KM)),
                        op=mult)
                    yrow = p1.tile([128, 64], f32, tag="yrow")
                    nc.vector.reduce_sum(
                        yrow[:, 0:COUT].unsqueeze(2),
                        o1[:].rearrange("p (o j) -> p o j", j=KM), axis=X)
                    nc.sync.dma_start(
                        ycat_d.ap()[b * 128:(b + 1) * 128, 0:COUT], yrow[:, 0:COUT])

            # ---------- sigma2 ----------
            NCH = 256 // S2CHUNK          # chunks
            GC = S2CHUNK * 4              # group-columns per chunk (G2FIX=4)
            with tc.tile_pool(name="ph2", bufs=2) as p2, \
                    tc.tile_pool(name="ps2", bufs=2, space="PSUM") as q2:
                tab2 = p2.tile([128, 8 * SG2], f16, tag="tab2", bufs=1)
                nc.sync.dma_start(tab2[:], tab2_d[:])
                tab23 = tab2[:].rearrange("p (t s) -> p t s", t=8)
                yi2 = p2.tile([128, SG2 * 8], i16, tag="yi2", bufs=1)
                nc.sync.dma_start(yi2[:], yidx2_d[:])
                for c in range(NCH):
                    s0 = c * GC           # first group-col of chunk
                    sl = slice(s0, s0 + GC)
                    gy = p2.tile([128, GC * 64], f32, tag="gy")
                    gy3 = gy[:].rearrange("p (g e) -> p g e", e=64)
                    nc.gpsimd.dma_gather(
                        gy3, ycat_d[:],
                        yi2[:, s0 * 8:(s0 + GC) * 8],
                        GC * 128, GC * 128, 64,
                        elem_step=64, single_packet=False)
                    dd = p2.tile([128, GC * 2], f32, tag="dd2")
                    dd3 = dd[:].rearrange("p (g t) -> p g t", t=2)
                    nc.vector.tensor_tensor(
                        dd3, tab23[:, 0:2, sl].rearrange("p t s -> p s t"),
                        tab23[:, 2:4, sl].rearrange("p t s -> p s t"), op=subtract)
                    nc.vector.tensor_tensor(dd3, dd3, dd3, op=mult)
                    nc.vector.tensor_tensor(
                        dd3, dd3,
                        tab23[:, 4:6, sl].rearrange("p t s -> p s t"), op=mult)
                    ga = p2.tile([128, GC], f32, tag="ga2")
                    nc.vector.tensor_tensor(ga[:], dd3[:, :, 0], dd3[:, :, 1],
                                            op=add)
                    nc.scalar.activation(ga[:], ga[:], Exp, scale=-1.0)
                    v2 = p2.tile([128, GC * 32], f16, tag="v2")
                    v23 = v2[:].rearrange("p (g e) -> p g e", e=32)
                    nc.vector.tensor_tensor(
                        v23, gy3[:, :, 0:32],
                        ga[:].unsqueeze(2).broadcast_to((128, GC, 32)), op=mult)
                    # one-hots: separate planes for even/odd target halves
                    ohe = p2.tile([128, GC * 128], f16, tag="ohe")
                    ohe3 = ohe[:].rearrange("p (g e) -> p g e", e=128)
                    nc.vector.tensor_tensor(
                        ohe3,
                        tab23[:, 6, sl].unsqueeze(2).broadcast_to((128, GC, 128)),
                        iota[:].unsqueeze(1).broadcast_to((128, GC, 128)),
                        op=is_equal)
                    oho = p2.tile([128, GC * 128], f16, tag="oho")
                    oho3 = oho[:].rearrange("p (g e) -> p g e", e=128)
                    nc.vector.tensor_tensor(
                        oho3,
                        tab23[:, 7, sl].unsqueeze(2).broadcast_to((128, GC, 128)),
                        iota[:].unsqueeze(1).broadcast_to((128, GC, 128)),
                        op=is_equal)
                    ob = p2.tile([128, S2CHUNK * 64], f32, tag="ob")
                    ob3 = ob[:].rearrange("p (k e) -> p k e", e=64)
                    for k in range(S2CHUNK):
                        po = q2.tile([128, 64], f32, tag="po")
                        po3 = po[:].rearrange("p (h e) -> p h e", e=32)
                        for g in range(4):
                            gc = 4 * k + g
                            nc.tensor.matmul(po3[:, 0, :], ohe3[:, gc, :],
                                             v23[:, gc, :],
                                             start=(g == 0), stop=(g == 3))
                        for g in range(4):
                            gc = 4 * k + g
                            nc.tensor.matmul(po3[:, 1, :], oho3[:, gc, :],
                                             v23[:, gc, :],
                                             start=(g == 0), stop=(g == 3))
                        nc.vector.tensor_copy(ob3[:, k, :], po[:])
                    nc.sync.dma_start(
                        out_d.ap()[c * S2CHUNK * 128:(c + 1) * S2CHUNK * 128, :]
                        .rearrange("(k p) e -> p k e", p=128),
                        ob3)
    return nc


def make_in_maps(cfg, x, grid, grid_weight, edge_grid, edge_Gauss, basepts,
                 base_weight, D, weights):
    maps, invs = [], []
    for b in range(x.shape[0]):
        m, inv2 = host_prep(cfg, x[b], grid[b], grid_weight[b], edge_grid[b],
                            edge_Gauss[b], basepts, base_weight, D, weights)
        maps.append(m)
        invs.append(inv2)
    return maps, invs


def finish(cfg, out_tbl, p_newrow):
    # device row p_newrow[pair] holds pair's output
    o = out_tbl[p_newrow]
    return np.ascontiguousarray(
        o.reshape(cfg["N"], 32)[:, :cfg["COUT"]].T)


_BUILT = {}


def _get_nc():
    if "nc" not in _BUILT:
        nc = bacc.Bacc("TRN2", target_bir_lowering=False,
                       dynamic_dma_scratch_size=32768)
        build(nc, CFG)
        nc.compile()
        _BUILT["nc"] = nc
    return _BUILT["nc"]


def kernel(x, grid, grid_weight, edge_grid, edge_Gauss, basepts, base_weight,
           D, weights, _trace=False):
    cfg = CFG
    in_maps, invs = make_in_maps(
        cfg, np.asarray(x, np.float32), np.asarray(grid),
        np.asarray(grid_weight), np.asarray(edge_grid),
        np.asarray(edge_Gauss), np.asarray(basepts),
        np.asarray(base_weight), np.asarray(D), np.asarray(weights))
    nc = _get_nc()
    res = bass_utils.run_bass_kernel_spmd(
        nc, in_maps, core_ids=list(range(x.shape[0])), trace=_trace)
    out = np.stack([finish(cfg, res.results[b]["out"], invs[b])
                    for b in range(x.shape[0])])
    kernel.last_result = res
    return out


# revision 18
# speedup vs baseline: 3.8435x; 1.6035x over previous
"""GPDconv (GNN message passing) Trainium2 Bass kernel — PE one-hot design.

Batch-parallel over 8 NeuronCores (one batch per core). The previous design
spent ~4ms/core in Q7 SWDGE descriptor generation (~8ns per gather index,
~500k indices). This version keeps exactly TWO per-edge SWDGE passes (the
provable floor) and does all aggregation on the PE via one-hot matmuls:

  sigma1: edges sorted into 32 host-balanced target-blocks (128 ega-targets,
    exactly 4096 edges each). One dma_gather of x pair-rows per edge
    (+ ~6% slot padding from the rnorm partition constraint). Per 128-edge
    group: V1 = u*rnorm*x_row, one-hot over within-block target -> PE matmul
    accumulating x_hat^T [32ch, 128t] in PSUM. rnorm[p] is delivered by a
    96-plane select: edge partition q == (p + rot_c) % 128 for one of three
    rotations (3-choice load balancing), rnorm planes live at [q, 32c+j].
  C: y = (x_hat @ W) . D^T per 128-target tile (targets in permuted order).
  sigma2: edges sorted into 256 host-balanced pair-blocks (128 node-pairs,
    exactly 512 edges each). One dma_gather of y rows per edge (zero pad).
    V2 = gauss*(parity masks)*y, one-hot over within-block pair -> PE matmul
    -> out pair-rows [128, 64] per block, written permuted; host unpermutes.

Host does index/layout prep only (sorting, balancing, packing, int16);
all value math (gauss, norms, products, reductions) runs on device.
"""
import sys

if '/opt/trn_rl_repo' not in sys.path:
    sys.path.insert(0, '/opt/trn_rl_repo')

import numpy as np
import concourse.bacc as bacc
import concourse.mybir as mybir
import concourse.tile as tile
from concourse import bass_utils, library_config, masks

f32 = mybir.dt.float32
f16 = mybir.dt.float16
i16 = mybir.dt.int16

CFG = dict(N=65536, NUM_PTS=4096, K=32, CIN=32, COUT=32, KM=16,
           G1FIX=34, ROTS=(0, 43, 86), S2CHUNK=8)

mult, add, subtract = (mybir.AluOpType.mult, mybir.AluOpType.add,
                       mybir.AluOpType.subtract)
is_equal = mybir.AluOpType.is_equal
Exp = mybir.ActivationFunctionType.Exp
X = mybir.AxisListType.X


def _wrap16(a):
    return np.ascontiguousarray(np.tile(a.reshape(-1, 16).T, (8, 1)))


def _balance_blocks(deg, nblocks, per_block_items, per_block_sum):
    """Partition items into nblocks of exactly per_block_items items with
    degree sums exactly per_block_sum. Snake-deal + exact swap repair."""
    deg = np.asarray(deg, np.int64)
    n = len(deg)
    assert n == nblocks * per_block_items
    assert deg.sum() == nblocks * per_block_sum
    order = np.argsort(-deg, kind='stable')
    # snake deal: rows of nblocks, alternating direction
    rows = order.reshape(per_block_items, nblocks)
    for r in range(1, per_block_items, 2):
        rows[r] = rows[r][::-1]
    blocks = [list(rows[:, b]) for b in range(nblocks)]
    sums = np.array([deg[b].sum() for b in blocks], np.int64)
    for _ in range(100000):
        dev = sums - per_block_sum
        if not dev.any():
            break
        hi = int(np.argmax(dev))
        lo = int(np.argmin(dev))
        dstar = int(min(dev[hi], -dev[lo]))
        ha = np.asarray(blocks[hi])
        la = np.asarray(blocks[lo])
        da, db = deg[ha], deg[la]
        ua = np.unique(da)
        ub = np.unique(db)
        found = None
        for want in range(dstar, 0, -1):
            hit = ua[np.isin(ua - want, ub)]
            if len(hit):
                va = int(hit[0])
                ai = int(np.nonzero(da == va)[0][0])
                bj = int(np.nonzero(db == va - want)[0][0])
                found = (ai, bj, want)
                break
        assert found is not None, (dev[hi], dev[lo], ua, ub)
        ai, bj, want = found
        a_it, b_it = int(ha[ai]), int(la[bj])
        blocks[hi][ai] = b_it
        blocks[lo][bj] = a_it
        sums[hi] -= want
        sums[lo] += want
    assert (sums == per_block_sum).all(), sums
    return [np.asarray(b, np.int64) for b in blocks]


def _assign_bins(res, rots, cap):
    """3-choice capacitated assignment: edge i may go to bin
    (res[i]+rot)%128; return bin per edge with loads <= cap.
    Greedy lightest-bin init + BFS augmenting-path eviction."""
    n = len(res)
    nr = len(rots)
    cands = np.stack([(res + r) % 128 for r in rots], 1)   # (n, nr)
    cnt = np.zeros(128, np.int64)
    choice = np.zeros(n, np.int64)
    order = np.random.default_rng(0).permutation(n)
    for i in order:
        c = cands[i]
        j = int(np.argmin(cnt[c]))
        choice[i] = j
        cnt[c[j]] += 1
    # bin -> member edge list
    members = [[] for _ in range(128)]
    for i in range(n):
        members[int(cands[i, choice[i]])].append(i)
    while True:
        over = [b for b in range(128) if cnt[b] > cap]
        if not over:
            break
        s = over[0]
        # BFS from s to any bin with load < cap via edge reassignments
        parent = {s: None}
        frontier = [s]
        goal = None
        while frontier and goal is None:
            nxt = []
            for u in frontier:
                for i in members[u]:
                    for j in range(nr):
                        v = int(cands[i, j])
                        if v == u or v in parent:
                            continue
                        parent[v] = (u, i, j)
                        if cnt[v] < cap:
                            goal = v
                            break
                        nxt.append(v)
                    if goal is not None:
                        break
                if goal is not None:
                    break
            frontier = nxt
        assert goal is not None, "no augmenting path; raise G1FIX"
        # walk back, reassigning one edge per hop
        v = goal
        while parent[v] is not None:
            u, i, j = parent[v]
            members[u].remove(i)
            members[v].append(i)
            choice[i] = j
            cnt[u] -= 1
            cnt[v] += 1
            v = u
    assert cnt.max() <= cap, (cnt.max(), cap)
    return cands[np.arange(n), choice]


def host_prep(cfg, x_b, grid_b, gw_b, eg_b, ega_b, basepts, base_weight, D,
              weights):
    N, NUM_PTS, K = cfg["N"], cfg["NUM_PTS"], cfg["K"]
    CIN, COUT, KM = cfg["CIN"], cfg["COUT"], cfg["KM"]
    G1FIX, ROTS = cfg["G1FIX"], cfg["ROTS"]
    E = K * NUM_PTS
    PCOLS = NUM_PTS // 128
    eg = eg_b.T.reshape(-1).astype(np.int64)        # (E,) [k, p] order
    ega = ega_b.T.reshape(-1).astype(np.int64)
    pp = np.tile(np.arange(NUM_PTS), K)

    # ---------------- xcat pair-row table ----------------
    rows = np.zeros((N, 64), np.float32)
    rows[:, :CIN] = x_b.T
    rows[:, CIN] = grid_b[:, 0]
    rows[:, CIN + 1] = grid_b[:, 1]
    rows[:, CIN + 2] = gw_b
    xcat = rows.astype(np.float16).reshape(N // 2, 128)

    # ---------------- dense tab (rnorm pass) ----------------
    def lay_dense(v):
        return np.ascontiguousarray(
            v.reshape(K, PCOLS, 128).transpose(2, 1, 0).reshape(128, E // 128))
    dtab = np.stack([
        lay_dense(grid_b[eg, 0].reshape(K, NUM_PTS)),
        lay_dense(grid_b[eg, 1].reshape(K, NUM_PTS)),
        lay_dense(gw_b[eg].reshape(K, NUM_PTS)),
        lay_dense(basepts[ega, 0].reshape(K, NUM_PTS)),
        lay_dense(basepts[ega, 1].reshape(K, NUM_PTS)),
    ], axis=-1).astype(np.float16)
    bwd = np.stack([base_weight[:, 0].reshape(PCOLS, 128).T,
                    base_weight[:, 1].reshape(PCOLS, 128).T], axis=-1)

    # ---------------- sigma1: balanced target blocks ----------------
    tdeg = np.bincount(ega, minlength=NUM_PTS)
    blocks1 = _balance_blocks(tdeg, 32, 128, E // 32)
    t_newrow = np.empty(NUM_PTS, np.int64)          # orig target -> new row
    t_local = np.empty(NUM_PTS, np.int64)
    t_block = np.empty(NUM_PTS, np.int64)
    for b in range(32):
        t_newrow[blocks1[b]] = 128 * b + np.arange(128)
        t_local[blocks1[b]] = np.arange(128)
        t_block[blocks1[b]] = b

    SG1 = 32 * G1FIX
    S1 = SG1 * 128
    xidx1 = np.zeros(S1, np.int16)
    tab1 = np.zeros((S1, 8), np.float16)            # bpx bpy bwx bwy me mo egar prow
    tab1[:, 6] = -1.0
    tab1[:, 7] = 127.0                              # no plane match for holes
    for b in range(32):
        sel = np.nonzero(t_block[ega] == b)[0]
        assert len(sel) == E // 32
        res = pp[sel] % 128
        q = _assign_bins(res, ROTS, G1FIX)
        # slot within block: (q, g) with g = rank within bin q
        order = np.argsort(q, kind='stable')
        sel, q = sel[order], q[order]
        cnt = np.bincount(q, minlength=128)
        starts = np.concatenate([[0], np.cumsum(cnt)])[:-1]
        g = np.arange(len(sel)) - starts[q]
        slot = (b * G1FIX + g) * 128 + q
        xidx1[slot] = (eg[sel] >> 1).astype(np.int16)
        tab1[slot, 0] = basepts[ega[sel], 0]
        tab1[slot, 1] = basepts[ega[sel], 1]
        tab1[slot, 2] = base_weight[pp[sel], 0]
        tab1[slot, 3] = base_weight[pp[sel], 1]
        tab1[slot, 4] = (1 - (eg[sel] & 1)).astype(np.float16)
        tab1[slot, 5] = (eg[sel] & 1).astype(np.float16)
        tab1[slot, 6] = t_local[ega[sel]].astype(np.float16)
        rot_used = (q - pp[sel]) % 128
        cidx = np.zeros(len(sel), np.int64)
        for ci, r in enumerate(ROTS):
            cidx[rot_used == r] = ci
        tab1[slot, 7] = (cidx * 32 + (pp[sel] >> 7)).astype(np.float16)

    # tab1 device layout: [128, 8, SG1] (plane-major per partition)
    tab1_dev = np.ascontiguousarray(
        tab1.reshape(SG1, 128, 8).transpose(1, 2, 0)).astype(np.float16)

    # rotation matrices for rnorm planes (f16): R[q, q'] = [q' == (q+rot)%128]
    rotm = np.zeros((2, 128, 128), np.float16)
    for ci, r in enumerate(ROTS[1:]):
        rotm[ci, np.arange(128), (np.arange(128) + r) % 128] = 1.0

    # ---------------- sigma2: balanced pair blocks ----------------
    m2 = eg >> 1
    pdeg = np.bincount(m2, minlength=N // 2)
    blocks2 = _balance_blocks(pdeg, 256, 128, E // 256)
    p_local = np.empty(N // 2, np.int64)
    p_block = np.empty(N // 2, np.int64)
    p_newrow = np.empty(N // 2, np.int64)
    for b in range(256):
        p_local[blocks2[b]] = np.arange(128)
        p_block[blocks2[b]] = b
        p_newrow[blocks2[b]] = 128 * b + np.arange(128)

    SG2 = 1024
    S2 = SG2 * 128
    yidx2 = np.zeros(S2, np.int16)
    tab2 = np.zeros((S2, 8), np.float16)            # gx gy bpx bpy bwx bwy gme gmo... see below
    tab2[:, 7] = -1.0                               # prel hole marker unused (masks=0)
    slot2_of = np.empty(E, np.int64)
    pos = 0
    for b in range(256):
        sel = np.nonzero(p_block[m2] == b)[0]
        assert len(sel) == E // 256
        n = len(sel)
        slot = pos + np.arange(n)
        pos += n
        yidx2[slot] = t_newrow[ega[sel]].astype(np.int16)
        tab2[slot, 0] = grid_b[eg[sel], 0]
        tab2[slot, 1] = grid_b[eg[sel], 1]
        tab2[slot, 2] = basepts[ega[sel], 0]
        tab2[slot, 3] = basepts[ega[sel], 1]
        tab2[slot, 4] = base_weight[pp[sel], 0]
        tab2[slot, 5] = base_weight[pp[sel], 1]
        # fold parity into masks; 6 = even mask, 7 = prel; odd mask = 1-even... we
        # need BOTH me and mo plus prel: pack me in plane 6 and prel+256*mo?? no:
        # use plane 6 = prel for even-half, plane 7 = prel for odd-half: the two
        # V2 halves use separate one-hots? cheaper: keep me/mo implicit:
        # prel_even = prel if even else -1; prel_odd = prel if odd else -1.
        prel = p_local[m2[sel]].astype(np.float16)
        odd = (eg[sel] & 1).astype(bool)
        pe = prel.copy(); pe[odd] = -1.0
        po = prel.copy(); po[~odd] = -1.0
        tab2[slot, 6] = pe
        tab2[slot, 7] = po
        slot2_of[sel] = slot
    tab2_dev = np.ascontiguousarray(
        tab2.reshape(SG2, 128, 8).transpose(1, 2, 0)).astype(np.float16)

    # host finish: orig pair row = out_tbl[p_newrow[pair]]

    # dtt rows permuted by target new-row
    t_origin = np.empty(NUM_PTS, np.int64)
    t_origin[t_newrow] = np.arange(NUM_PTS)
    dtt = np.ascontiguousarray(D.T[t_origin].astype(np.float32))

    iota_row = np.tile(np.arange(128, dtype=np.float16)[None, :], (128, 1))

    return dict(
        xcat=xcat,
        dtab=dtab,
        bwd=np.ascontiguousarray(bwd.astype(np.float32)),
        wfl=np.ascontiguousarray(weights.reshape(CIN, COUT * KM).astype(np.float32)),
        dt_t=dtt,
        xidx1=_wrap16(xidx1),
        tab1=tab1_dev.reshape(128, 8 * SG1),
        rotm=np.ascontiguousarray(rotm.reshape(2 * 128, 128)),
        yidx2=_wrap16(yidx2),
        tab2=tab2_dev.reshape(128, 8 * SG2),
        iota=np.ascontiguousarray(iota_row),
    ), p_newrow


def build(nc, cfg):
    N, NUM_PTS, K = cfg["N"], cfg["NUM_PTS"], cfg["K"]
    CIN, COUT, KM = cfg["CIN"], cfg["COUT"], cfg["KM"]
    G1FIX = cfg["G1FIX"]
    S2CHUNK = cfg["S2CHUNK"]
    E = K * NUM_PTS
    PCOLS = NUM_PTS // 128
    OJ = COUT * KM
    SG1 = 32 * G1FIX
    SG2 = 1024
    NPLANES = 96

    xcat_d = nc.dram_tensor("xcat", [N // 2, 128], f16, kind="ExternalInput")
    dtab_d = nc.dram_tensor("dtab", [128, E // 128, 5], f16, kind="ExternalInput")
    bwd_d = nc.dram_tensor("bwd", [128, PCOLS, 2], f32, kind="ExternalInput")
    wfl_d = nc.dram_tensor("wfl", [CIN, OJ], f32, kind="ExternalInput")
    dtt_d = nc.dram_tensor("dt_t", [NUM_PTS, KM], f32, kind="ExternalInput")
    xidx1_d = nc.dram_tensor("xidx1", [128, SG1 * 128 // 16], i16, kind="ExternalInput")
    tab1_d = nc.dram_tensor("tab1", [128, 8 * SG1], f16, kind="ExternalInput")
    rotm_d = nc.dram_tensor("rotm", [2 * 128, 128], f16, kind="ExternalInput")
    yidx2_d = nc.dram_tensor("yidx2", [128, SG2 * 128 // 16], i16, kind="ExternalInput")
    tab2_d = nc.dram_tensor("tab2", [128, 8 * SG2], f16, kind="ExternalInput")
    iota_d = nc.dram_tensor("iota", [128, 128], f16, kind="ExternalInput")
    out_d = nc.dram_tensor("out", [N // 2, 64], f32, kind="ExternalOutput")
    ycat_d = nc.dram_tensor("ycat_tbl", [NUM_PTS, 64], f32, kind="Internal")

    with tile.TileContext(nc) as tc:
        with tc.tile_pool(name="consts", bufs=1) as cp:
            ident = cp.tile([128, 128], f32)
            masks.make_identity(nc, ident[:])
            nc.gpsimd.load_library(library_config.mlp)

            wfl = cp.tile([CIN, OJ], f32)
            nc.sync.dma_start(wfl[:], wfl_d[:])
            bwd = cp.tile([128, PCOLS * 2], f32)
            bwd3 = bwd[:].rearrange("p (q t) -> p q t", t=2)
            nc.sync.dma_start(bwd3, bwd_d[:])
            iota = cp.tile([128, 128], f16)
            nc.sync.dma_start(iota[:], iota_d[:])
            rotm = cp.tile([128, 2 * 128], f16)
            nc.sync.dma_start(rotm[:].rearrange("p (c j) -> p c j", c=2),
                              rotm_d.ap().rearrange("(c p) j -> p c j", p=128))
            tab1 = cp.tile([128, 8 * SG1], f16)
            nc.sync.dma_start(tab1[:], tab1_d[:])
            tab13 = tab1[:].rearrange("p (t s) -> p t s", t=8)
            xi1 = cp.tile([128, SG1 * 8], i16)
            nc.sync.dma_start(xi1[:], xidx1_d[:])


            # ---------- dense pass: rnorm planes (cp pool: no SBUF reuse
            # so sigma1 gathers are not blocked behind this) ----------
            rnt = cp.tile([128, NPLANES], f16)
            with tc.tile_pool(name="dpsum", bufs=1, space="PSUM") as dq:
                JD = E // 128
                dtab = cp.tile([128, JD * 5], f16)
                dt3 = dtab[:].rearrange("p (j t) -> p j t", t=5)
                nc.sync.dma_start(dt3, dtab_d[:])
                dd0 = cp.tile([128, JD], f32)
                dd1 = cp.tile([128, JD], f32)
                nc.vector.tensor_tensor(dd0[:], dt3[:, :, 0], dt3[:, :, 3], op=subtract)
                nc.vector.tensor_tensor(dd0[:], dd0[:], dd0[:], op=mult)
                nc.vector.tensor_tensor(dd1[:], dt3[:, :, 1], dt3[:, :, 4], op=subtract)
                nc.vector.tensor_tensor(dd1[:], dd1[:], dd1[:], op=mult)
                d0k = dd0[:].rearrange("p (q k) -> p q k", k=K)
                d1k = dd1[:].rearrange("p (q k) -> p q k", k=K)
                nc.vector.tensor_tensor(d0k, d0k,
                                        bwd3[:, :, 0].broadcast_to((128, PCOLS, K)),
                                        op=mult)
                nc.vector.tensor_tensor(d1k, d1k,
                                        bwd3[:, :, 1].broadcast_to((128, PCOLS, K)),
                                        op=mult)
                nc.vector.tensor_tensor(dd0[:], dd0[:], dd1[:], op=add)
                nc.scalar.activation(dd1[:], dd0[:], Exp, scale=-1.0)
                nc.vector.tensor_tensor(dd1[:], dd1[:], dt3[:, :, 2], op=mult)
                nc.vector.tensor_tensor(dd1[:], dd1[:], dd1[:], op=mult)
                nsq = cp.tile([128, PCOLS], f32)
                nc.vector.reduce_sum(nsq[:].unsqueeze(2),
                                     dd1[:].rearrange("p (q k) -> p q k", k=K),
                                     axis=X)
                nc.scalar.activation(nsq[:], nsq[:],
                                     mybir.ActivationFunctionType.Sqrt)
                nc.vector.tensor_scalar_add(nsq[:], nsq[:], 1e-5)
                nc.vector.reciprocal(nsq[:], nsq[:])
                nc.vector.tensor_copy(rnt[:, 0:32], nsq[:])
                for ci in range(2):
                    rp = dq.tile([128, 32], f32, tag="rp")
                    nc.tensor.matmul(rp[:], rotm[:, ci * 128:(ci + 1) * 128],
                                     rnt[:, 0:32], start=True, stop=True)
                    nc.vector.tensor_copy(rnt[:, 32 + 32 * ci:64 + 32 * ci], rp[:])

            # ---------- rn_all: 96-plane select (4 chunks so sigma1's early
            # blocks unblock before the whole select finishes) ----------
            rn_all = cp.tile([128, SG1], f16)
            rtmp = cp.tile([128, SG1], f16)
            nc.vector.memset(rn_all[:], 0.0)
            prow = tab13[:, 7, :]
            RC = SG1 // 4
            for r in range(4):
                cs = slice(r * RC, (r + 1) * RC)
                for j in range(NPLANES):
                    nc.vector.scalar_tensor_tensor(
                        rtmp[:, cs], prow[:, cs], float(j),
                        rnt[:, j:j + 1].broadcast_to((128, RC)),
                        op0=is_equal, op1=mult)
                    nc.vector.tensor_tensor(rn_all[:, cs], rn_all[:, cs],
                                            rtmp[:, cs], op=add)

            # ---------- sigma1 + fused phase C ----------
            xhT = cp.tile([CIN, NUM_PTS], f32)
            with tc.tile_pool(name="ph1", bufs=3) as p1, \
                    tc.tile_pool(name="ps1", bufs=2, space="PSUM") as q1:
                for b in range(32):
                    sl = slice(b * G1FIX, (b + 1) * G1FIX)
                    gx = p1.tile([128, G1FIX * 128], f16, tag="gx", bufs=5)
                    gx3 = gx[:].rearrange("p (g e) -> p g e", e=128)
                    nc.gpsimd.dma_gather(
                        gx3, xcat_d[:],
                        xi1[:, b * G1FIX * 8:(b + 1) * G1FIX * 8],
                        G1FIX * 128, G1FIX * 128, 128,
                        elem_step=128, single_packet=False,
                        queue_num=b % 4)
                    me = tab13[:, 4, sl]
                    mo = tab13[:, 5, sl]
                    # grid/gw of the edge's node via parity select
                    ge = p1.tile([128, G1FIX * 3], f32, tag="ge")
                    ge3 = ge[:].rearrange("p (g t) -> p g t", t=3)
                    t0 = p1.tile([128, G1FIX * 3], f32, tag="t0")
                    t03 = t0[:].rearrange("p (g t) -> p g t", t=3)
                    nc.vector.tensor_tensor(
                        ge3, gx3[:, :, 32:35],
                        me.unsqueeze(2).broadcast_to((128, G1FIX, 3)), op=mult)
                    nc.vector.tensor_tensor(
                        t03, gx3[:, :, 96:99],
                        mo.unsqueeze(2).broadcast_to((128, G1FIX, 3)), op=mult)
                    nc.vector.tensor_tensor(ge3, ge3, t03, op=add)
                    dd = p1.tile([128, G1FIX * 2], f32, tag="dd")
                    dd3 = dd[:].rearrange("p (g t) -> p g t", t=2)
                    nc.vector.tensor_tensor(
                        dd3, ge3[:, :, 0:2],
                        tab13[:, 0:2, sl].rearrange("p t s -> p s t"), op=subtract)
                    nc.vector.tensor_tensor(dd3, dd3, dd3, op=mult)
                    nc.vector.tensor_tensor(
                        dd3, dd3,
                        tab13[:, 2:4, sl].rearrange("p t s -> p s t"), op=mult)
                    ga = p1.tile([128, G1FIX], f32, tag="ga")
                    nc.vector.tensor_tensor(ga[:], dd3[:, :, 0], dd3[:, :, 1],
                                            op=add)
                    nc.scalar.activation(ga[:], ga[:], Exp, scale=-1.0)
                    nc.vector.tensor_tensor(ga[:], ga[:], ge3[:, :, 2], op=mult)
                    nc.vector.tensor_tensor(ga[:], ga[:], rn_all[:, sl], op=mult)
                    wlo = p1.tile([128, G1FIX], f32, tag="wlo")
                    whi = p1.tile([128, G1FIX], f32, tag="whi")
                    nc.vector.tensor_tensor(wlo[:], ga[:], me, op=mult)
                    nc.vector.tensor_tensor(whi[:], ga[:], mo, op=mult)
                    v1 = p1.tile([128, G1FIX * CIN], f16, tag="v1")
                    v13 = v1[:].rearrange("p (g e) -> p g e", e=CIN)
                    t1 = p1.tile([128, G1FIX * CIN], f16, tag="t1")
                    t13 = t1[:].rearrange("p (g e) -> p g e", e=CIN)
                    nc.vector.tensor_tensor(
                        v13, gx3[:, :, 0:CIN],
                        wlo[:].unsqueeze(2).broadcast_to((128, G1FIX, CIN)),
                        op=mult)
                    nc.vector.tensor_tensor(
                        t13, gx3[:, :, 64:64 + CIN],
                        whi[:].unsqueeze(2).broadcast_to((128, G1FIX, CIN)),
                        op=mult)
                    nc.vector.tensor_tensor(v13, v13, t13, op=add)
                    oh = p1.tile([128, G1FIX * 128], f16, tag="oh")
                    oh3 = oh[:].rearrange("p (g e) -> p g e", e=128)
                    nc.vector.tensor_tensor(
                        oh3,
                        tab13[:, 6, sl].unsqueeze(2).broadcast_to((128, G1FIX, 128)),
                        iota[:].unsqueeze(1).broadcast_to((128, G1FIX, 128)),
                        op=is_equal)
                    ps = q1.tile([CIN, 128], f32, tag="pxh")
                    for g in range(G1FIX):
                        nc.tensor.matmul(ps[:], v13[:, g, :], oh3[:, g, :],
                                         start=(g == 0), stop=(g == G1FIX - 1))
                    nc.vector.tensor_copy(xhT[:, b * 128:(b + 1) * 128], ps[:])
                    # fused phase C for this 128-target tile
                    o1p = q1.tile([128, OJ], f32, tag="o1p")
                    nc.tensor.matmul(o1p[:], xhT[:, b * 128:(b + 1) * 128],
                                     wfl[:], start=True, stop=True)
                    dtt = p1.tile([128, KM], f32, tag="dtt")
                    nc.sync.dma_start(dtt[:], dtt_d[b * 128:(b + 1) * 128, :])
                    o1 = p1.tile([128, OJ], f32, tag="o1")
                    nc.vector.tensor_tensor(
                        o1[:].rearrange("p (o j) -> p o j", j=KM),
                        o1p[:].rearrange("p (o j) -> p o j", j=KM),
                        dtt[:].unsqueeze(1).broadcast_to((128, COUT, KM)),
                        op=mult)
                    yrow = p1.tile([128, 64], f32, tag="yrow")
                    nc.vector.reduce_sum(
                        yrow[:, 0:COUT].unsqueeze(2),
                        o1[:].rearrange("p (o j) -> p o j", j=KM), axis=X)
                    nc.sync.dma_start(
                        ycat_d.ap()[b * 128:(b + 1) * 128, 0:COUT], yrow[:, 0:COUT])

            # ---------- sigma2 ----------
            NCH = 256 // S2CHUNK          # chunks
            GC = S2CHUNK * 4              # group-columns per chunk (G2FIX=4)
            with tc.tile_pool(name="ph2", bufs=2) as p2, \
                    tc.tile_pool(name="ps2", bufs=2, space="PSUM") as q2:
                tab2 = p2.tile([128, 8 * SG2], f16, tag="tab2", bufs=1)
                nc.sync.dma_start(tab2[:], tab2_d[:])
                tab23 = tab2[:].rearrange("p (t s) -> p t s", t=8)
                yi2 = p2.tile([128, SG2 * 8], i16, tag="yi2", bufs=1)
                nc.sync.dma_start(yi2[:], yidx2_d[:])
                for c in range(NCH):
                    s0 = c * GC           # first group-col of chunk
                    sl = slice(s0, s0 + GC)
                    gy = p2.tile([128, GC * 64], f32, tag="gy")
                    gy3 = gy[:].rearrange("p (g e) -> p g e", e=64)
                    nc.gpsimd.dma_gather(
                        gy3, ycat_d[:],
                        yi2[:, s0 * 8:(s0 + GC) * 8],
                        GC * 128, GC * 128, 64,
                        elem_step=64, single_packet=False,
                        queue_num=c % 4)
                    dd = p2.tile([128, GC * 2], f32, tag="dd2")
                    dd3 = dd[:].rearrange("p (g t) -> p g t", t=2)
                    nc.vector.tensor_tensor(
                        dd3, tab23[:, 0:2, sl].rearrange("p t s -> p s t"),
                        tab23[:, 2:4, sl].rearrange("p t s -> p s t"), op=subtract)
                    nc.vector.tensor_tensor(dd3, dd3, dd3, op=mult)
                    nc.vector.tensor_tensor(
                        dd3, dd3,
                        tab23[:, 4:6, sl].rearrange("p t s -> p s t"), op=mult)
                    ga = p2.tile([128, GC], f32, tag="ga2")
                    nc.vector.tensor_tensor(ga[:], dd3[:, :, 0], dd3[:, :, 1],
                                            op=add)
                    nc.scalar.activation(ga[:], ga[:], Exp, scale=-1.0)
                    v2 = p2.tile([128, GC * 32], f16, tag="v2")
                    v23 = v2[:].rearrange("p (g e) -> p g e", e=32)
                    nc.vector.tensor_tensor(
                        v23, gy3[:, :, 0:32],
                        ga[:].unsqueeze(2).broadcast_to((128, GC, 32)), op=mult)
                    # one-hots: separate planes for even/odd target halves
                    ohe = p2.tile([128, GC * 128], f16, tag="ohe")
                    ohe3 = ohe[:].rearrange("p (g e) -> p g e", e=128)
                    nc.vector.tensor_tensor(
                        ohe3,
                        tab23[:, 6, sl].unsqueeze(2).broadcast_to((128, GC, 128)),
                        iota[:].unsqueeze(1).broadcast_to((128, GC, 128)),
                        op=is_equal)
                    oho = p2.tile([128, GC * 128], f16, tag="oho")
                    oho3 = oho[:].rearrange("p (g e) -> p g e", e=128)
                    nc.vector.tensor_tensor(
                        oho3,
                        tab23[:, 7, sl].unsqueeze(2).broadcast_to((128, GC, 128)),
                        iota[:].unsqueeze(1).broadcast_to((128, GC, 128)),
                        op=is_equal)
                    ob = p2.tile([128, S2CHUNK * 64], f32, tag="ob")
                    ob3 = ob[:].rearrange("p (k e) -> p k e", e=64)
                    for k in range(S2CHUNK):
                        po = q2.tile([128, 64], f32, tag="po")
                        po3 = po[:].rearrange("p (h e) -> p h e", e=32)
                        for g in range(4):
                            gc = 4 * k + g
                            nc.tensor.matmul(po3[:, 0, :], ohe3[:, gc, :],
                                             v23[:, gc, :],
                                             start=(g == 0), stop=(g == 3))
                        for g in range(4):
                            gc = 4 * k + g
                            nc.tensor.matmul(po3[:, 1, :], oho3[:, gc, :],
                                             v23[:, gc, :],
                                             start=(g == 0), stop=(g == 3))
                        nc.vector.tensor_copy(ob3[:, k, :], po[:])
                    nc.sync.dma_start(
                        out_d.ap()[c * S2CHUNK * 128:(c + 1) * S2CHUNK * 128, :]
                        .rearrange("(k p) e -> p k e", p=128),
                        ob3)
    return nc


def make_in_maps(cfg, x, grid, grid_weight, edge_grid, edge_Gauss, basepts,
                 base_weight, D, weights):
    maps, invs = [], []
    for b in range(x.shape[0]):
        m, inv2 = host_prep(cfg, x[b], grid[b], grid_weight[b], edge_grid[b],
                            edge_Gauss[b], basepts, base_weight, D, weights)
        maps.append(m)
        invs.append(inv2)
    return maps, invs


def finish(cfg, out_tbl, p_newrow):
    # device row p_newrow[pair] holds pair's output
    o = out_tbl[p_newrow]
    return np.ascontiguousarray(
        o.reshape(cfg["N"], 32)[:, :cfg["COUT"]].T)


_BUILT = {}


def _get_nc():
    if "nc" not in _BUILT:
        nc = bacc.Bacc("TRN2", target_bir_lowering=False,
                       dynamic_dma_scratch_size=32768,
                       num_swdge_queues=4)
        build(nc, CFG)
        nc.compile()
        _BUILT["nc"] = nc
    return _BUILT["nc"]


def kernel(x, grid, grid_weight, edge_grid, edge_Gauss, basepts, base_weight,
           D, weights, _trace=False):
    cfg = CFG
    in_maps, invs = make_in_maps(
        cfg, np.asarray(x, np.float32), np.asarray(grid),
        np.asarray(grid_weight), np.asarray(edge_grid),
        np.asarray(edge_Gauss), np.asarray(basepts),
        np.asarray(base_weight), np.asarray(D), np.asarray(weights))
    nc = _get_nc()
    res = bass_utils.run_bass_kernel_spmd(
        nc, in_maps, core_ids=list(range(x.shape[0])), trace=_trace)
    out = np.stack([finish(cfg, res.results[b]["out"], invs[b])
                    for b in range(x.shape[0])])
    kernel.last_result = res
    return out


# revision 21
# speedup vs baseline: 4.5599x; 1.1864x over previous
"""GPDconv (GNN message passing) Trainium2 Bass kernel — PE one-hot design.

Batch-parallel over 8 NeuronCores (one batch per core). The previous design
spent ~4ms/core in Q7 SWDGE descriptor generation (~8ns per gather index,
~500k indices). This version keeps exactly TWO per-edge SWDGE passes (the
provable floor) and does all aggregation on the PE via one-hot matmuls:

  sigma1: edges sorted into 32 host-balanced target-blocks (128 ega-targets,
    exactly 4096 edges each). One dma_gather of x pair-rows per edge
    (+ ~6% slot padding from the rnorm partition constraint). Per 128-edge
    group: V1 = u*rnorm*x_row, one-hot over within-block target -> PE matmul
    accumulating x_hat^T [32ch, 128t] in PSUM. rnorm[p] is delivered by a
    96-plane select: edge partition q == (p + rot_c) % 128 for one of three
    rotations (3-choice load balancing), rnorm planes live at [q, 32c+j].
  C: y = (x_hat @ W) . D^T per 128-target tile (targets in permuted order).
  sigma2: edges sorted into 256 host-balanced pair-blocks (128 node-pairs,
    exactly 512 edges each). One dma_gather of y rows per edge (zero pad).
    V2 = gauss*(parity masks)*y, one-hot over within-block pair -> PE matmul
    -> out pair-rows [128, 64] per block, written permuted; host unpermutes.

Host does index/layout prep only (sorting, balancing, packing, int16);
all value math (gauss, norms, products, reductions) runs on device.
"""
import sys

if '/opt/trn_rl_repo' not in sys.path:
    sys.path.insert(0, '/opt/trn_rl_repo')

import numpy as np
import concourse.bacc as bacc
import concourse.mybir as mybir
import concourse.tile as tile
from concourse import bass_utils, library_config, masks

f32 = mybir.dt.float32
f16 = mybir.dt.float16
i16 = mybir.dt.int16

CFG = dict(N=65536, NUM_PTS=4096, K=32, CIN=32, COUT=32, KM=16,
           G1FIX=34, ROTS=(0, 43, 86), S2CHUNK=8)

mult, add, subtract = (mybir.AluOpType.mult, mybir.AluOpType.add,
                       mybir.AluOpType.subtract)
is_equal = mybir.AluOpType.is_equal
Exp = mybir.ActivationFunctionType.Exp
X = mybir.AxisListType.X


def _wrap16(a):
    return np.ascontiguousarray(np.tile(a.reshape(-1, 16).T, (8, 1)))


def _balance_blocks(deg, nblocks, per_block_items, per_block_sum):
    """Partition items into nblocks of exactly per_block_items items with
    degree sums exactly per_block_sum. Snake-deal + exact swap repair."""
    deg = np.asarray(deg, np.int64)
    n = len(deg)
    assert n == nblocks * per_block_items
    assert deg.sum() == nblocks * per_block_sum
    order = np.argsort(-deg, kind='stable')
    # snake deal: rows of nblocks, alternating direction
    rows = order.reshape(per_block_items, nblocks)
    for r in range(1, per_block_items, 2):
        rows[r] = rows[r][::-1]
    blocks = [list(rows[:, b]) for b in range(nblocks)]
    sums = np.array([deg[b].sum() for b in blocks], np.int64)
    for _ in range(100000):
        dev = sums - per_block_sum
        if not dev.any():
            break
        hi = int(np.argmax(dev))
        lo = int(np.argmin(dev))
        dstar = int(min(dev[hi], -dev[lo]))
        ha = np.asarray(blocks[hi])
        la = np.asarray(blocks[lo])
        da, db = deg[ha], deg[la]
        ua = np.unique(da)
        ub = np.unique(db)
        found = None
        for want in range(dstar, 0, -1):
            hit = ua[np.isin(ua - want, ub)]
            if len(hit):
                va = int(hit[0])
                ai = int(np.nonzero(da == va)[0][0])
                bj = int(np.nonzero(db == va - want)[0][0])
                found = (ai, bj, want)
                break
        assert found is not None, (dev[hi], dev[lo], ua, ub)
        ai, bj, want = found
        a_it, b_it = int(ha[ai]), int(la[bj])
        blocks[hi][ai] = b_it
        blocks[lo][bj] = a_it
        sums[hi] -= want
        sums[lo] += want
    assert (sums == per_block_sum).all(), sums
    return [np.asarray(b, np.int64) for b in blocks]


def _assign_bins(res, rots, cap):
    """3-choice capacitated assignment: edge i may go to bin
    (res[i]+rot)%128; return bin per edge with loads <= cap.
    Greedy lightest-bin init + BFS augmenting-path eviction."""
    n = len(res)
    nr = len(rots)
    cands = np.stack([(res + r) % 128 for r in rots], 1)   # (n, nr)
    cnt = np.zeros(128, np.int64)
    choice = np.zeros(n, np.int64)
    order = np.random.default_rng(0).permutation(n)
    for i in order:
        c = cands[i]
        j = int(np.argmin(cnt[c]))
        choice[i] = j
        cnt[c[j]] += 1
    # bin -> member edge list
    members = [[] for _ in range(128)]
    for i in range(n):
        members[int(cands[i, choice[i]])].append(i)
    while True:
        over = [b for b in range(128) if cnt[b] > cap]
        if not over:
            break
        s = over[0]
        # BFS from s to any bin with load < cap via edge reassignments
        parent = {s: None}
        frontier = [s]
        goal = None
        while frontier and goal is None:
            nxt = []
            for u in frontier:
                for i in members[u]:
                    for j in range(nr):
                        v = int(cands[i, j])
                        if v == u or v in parent:
                            continue
                        parent[v] = (u, i, j)
                        if cnt[v] < cap:
                            goal = v
                            break
                        nxt.append(v)
                    if goal is not None:
                        break
                if goal is not None:
                    break
            frontier = nxt
        assert goal is not None, "no augmenting path; raise G1FIX"
        # walk back, reassigning one edge per hop
        v = goal
        while parent[v] is not None:
            u, i, j = parent[v]
            members[u].remove(i)
            members[v].append(i)
            choice[i] = j
            cnt[u] -= 1
            cnt[v] += 1
            v = u
    assert cnt.max() <= cap, (cnt.max(), cap)
    return cands[np.arange(n), choice]


def host_prep(cfg, x_b, grid_b, gw_b, eg_b, ega_b, basepts, base_weight, D,
              weights):
    N, NUM_PTS, K = cfg["N"], cfg["NUM_PTS"], cfg["K"]
    CIN, COUT, KM = cfg["CIN"], cfg["COUT"], cfg["KM"]
    G1FIX, ROTS = cfg["G1FIX"], cfg["ROTS"]
    E = K * NUM_PTS
    PCOLS = NUM_PTS // 128
    eg = eg_b.T.reshape(-1).astype(np.int64)        # (E,) [k, p] order
    ega = ega_b.T.reshape(-1).astype(np.int64)
    pp = np.tile(np.arange(NUM_PTS), K)

    # ---------------- xcat pair-row table ----------------
    rows = np.zeros((N, 64), np.float32)
    rows[:, :CIN] = x_b.T
    rows[:, CIN] = grid_b[:, 0]
    rows[:, CIN + 1] = grid_b[:, 1]
    rows[:, CIN + 2] = gw_b
    xcat = rows.astype(np.float16).reshape(N // 2, 128)

    # ---------------- dense tab (rnorm pass) ----------------
    def lay_dense(v):
        return np.ascontiguousarray(
            v.reshape(K, PCOLS, 128).transpose(2, 1, 0).reshape(128, E // 128))
    dtab = np.stack([
        lay_dense(grid_b[eg, 0].reshape(K, NUM_PTS)),
        lay_dense(grid_b[eg, 1].reshape(K, NUM_PTS)),
        lay_dense(gw_b[eg].reshape(K, NUM_PTS)),
        lay_dense(basepts[ega, 0].reshape(K, NUM_PTS)),
        lay_dense(basepts[ega, 1].reshape(K, NUM_PTS)),
    ], axis=-1).astype(np.float16)
    bwd = np.stack([base_weight[:, 0].reshape(PCOLS, 128).T,
                    base_weight[:, 1].reshape(PCOLS, 128).T], axis=-1)

    # ---------------- sigma1: balanced target blocks ----------------
    tdeg = np.bincount(ega, minlength=NUM_PTS)
    blocks1 = _balance_blocks(tdeg, 32, 128, E // 32)
    t_newrow = np.empty(NUM_PTS, np.int64)          # orig target -> new row
    t_local = np.empty(NUM_PTS, np.int64)
    t_block = np.empty(NUM_PTS, np.int64)
    for b in range(32):
        t_newrow[blocks1[b]] = 128 * b + np.arange(128)
        t_local[blocks1[b]] = np.arange(128)
        t_block[blocks1[b]] = b

    SG1 = 32 * G1FIX
    S1 = SG1 * 128
    xidx1 = np.zeros(S1, np.int16)
    tab1 = np.zeros((S1, 8), np.float16)            # bpx bpy bwx bwy me mo egar prow
    tab1[:, 6] = -1.0
    tab1[:, 7] = 127.0                              # no plane match for holes
    for b in range(32):
        sel = np.nonzero(t_block[ega] == b)[0]
        assert len(sel) == E // 32
        res = pp[sel] % 128
        q = _assign_bins(res, ROTS, G1FIX)
        # slot within block: (q, g) with g = rank within bin q
        order = np.argsort(q, kind='stable')
        sel, q = sel[order], q[order]
        cnt = np.bincount(q, minlength=128)
        starts = np.concatenate([[0], np.cumsum(cnt)])[:-1]
        g = np.arange(len(sel)) - starts[q]
        slot = (b * G1FIX + g) * 128 + q
        xidx1[slot] = (eg[sel] >> 1).astype(np.int16)
        tab1[slot, 0] = basepts[ega[sel], 0]
        tab1[slot, 1] = basepts[ega[sel], 1]
        tab1[slot, 2] = base_weight[pp[sel], 0]
        tab1[slot, 3] = base_weight[pp[sel], 1]
        tab1[slot, 4] = (1 - (eg[sel] & 1)).astype(np.float16)
        tab1[slot, 5] = (eg[sel] & 1).astype(np.float16)
        tab1[slot, 6] = t_local[ega[sel]].astype(np.float16)
        rot_used = (q - pp[sel]) % 128
        cidx = np.zeros(len(sel), np.int64)
        for ci, r in enumerate(ROTS):
            cidx[rot_used == r] = ci
        tab1[slot, 7] = (cidx * 32 + (pp[sel] >> 7)).astype(np.float16)

    # tab1 device layout: [128, 8, SG1] (plane-major per partition)
    tab1_dev = np.ascontiguousarray(
        tab1.reshape(SG1, 128, 8).transpose(1, 2, 0)).astype(np.float16)

    # rotation matrices for rnorm planes (f16): R[q, q'] = [q' == (q+rot)%128]
    rotm = np.zeros((2, 128, 128), np.float16)
    for ci, r in enumerate(ROTS[1:]):
        rotm[ci, np.arange(128), (np.arange(128) + r) % 128] = 1.0

    # ---------------- sigma2: balanced pair blocks ----------------
    m2 = eg >> 1
    pdeg = np.bincount(m2, minlength=N // 2)
    blocks2 = _balance_blocks(pdeg, 256, 128, E // 256)
    p_local = np.empty(N // 2, np.int64)
    p_block = np.empty(N // 2, np.int64)
    p_newrow = np.empty(N // 2, np.int64)
    for b in range(256):
        p_local[blocks2[b]] = np.arange(128)
        p_block[blocks2[b]] = b
        p_newrow[blocks2[b]] = 128 * b + np.arange(128)

    SG2 = 1024
    S2 = SG2 * 128
    yidx2 = np.zeros(S2, np.int16)
    tab2 = np.zeros((S2, 8), np.float16)            # gx gy bpx bpy bwx bwy gme gmo... see below
    tab2[:, 7] = -1.0                               # prel hole marker unused (masks=0)
    slot2_of = np.empty(E, np.int64)
    pos = 0
    for b in range(256):
        sel = np.nonzero(p_block[m2] == b)[0]
        assert len(sel) == E // 256
        n = len(sel)
        slot = pos + np.arange(n)
        pos += n
        yidx2[slot] = t_newrow[ega[sel]].astype(np.int16)
        tab2[slot, 0] = grid_b[eg[sel], 0]
        tab2[slot, 1] = grid_b[eg[sel], 1]
        tab2[slot, 2] = basepts[ega[sel], 0]
        tab2[slot, 3] = basepts[ega[sel], 1]
        tab2[slot, 4] = base_weight[pp[sel], 0]
        tab2[slot, 5] = base_weight[pp[sel], 1]
        # plane 6 = prel (pair within block), plane 7 = even-node mask
        tab2[slot, 6] = p_local[m2[sel]].astype(np.float16)
        tab2[slot, 7] = (1 - (eg[sel] & 1)).astype(np.float16)
        slot2_of[sel] = slot
    tab2_dev = np.ascontiguousarray(
        tab2.reshape(SG2, 128, 8).transpose(1, 2, 0)).astype(np.float16)

    # host finish: orig pair row = out_tbl[p_newrow[pair]]

    # dtt rows permuted by target new-row
    t_origin = np.empty(NUM_PTS, np.int64)
    t_origin[t_newrow] = np.arange(NUM_PTS)
    dtt = np.ascontiguousarray(D.T[t_origin].astype(np.float32))

    # tiled iota: [128, G1FIX*128], content[q, g*128+j] = j (one materialized
    # copy per group column so one-hot is_eq needs no stride-0 inner operand)
    iota_row = np.tile(np.arange(128, dtype=np.float16)[None, None, :],
                       (128, G1FIX, 1)).reshape(128, G1FIX * 128)

    return dict(
        xcat=xcat,
        dtab=dtab,
        bwd=np.ascontiguousarray(bwd.astype(np.float32)),
        wfl=np.ascontiguousarray(weights.reshape(CIN, COUT * KM).astype(np.float32)),
        dt_t=dtt,
        xidx1=_wrap16(xidx1),
        tab1=tab1_dev.reshape(128, 8 * SG1),
        rotm=np.ascontiguousarray(rotm.reshape(2 * 128, 128)),
        yidx2=_wrap16(yidx2),
        tab2=tab2_dev.reshape(128, 8 * SG2),
        iota=np.ascontiguousarray(iota_row),
    ), p_newrow


def build(nc, cfg):
    N, NUM_PTS, K = cfg["N"], cfg["NUM_PTS"], cfg["K"]
    CIN, COUT, KM = cfg["CIN"], cfg["COUT"], cfg["KM"]
    G1FIX = cfg["G1FIX"]
    S2CHUNK = cfg["S2CHUNK"]
    E = K * NUM_PTS
    PCOLS = NUM_PTS // 128
    OJ = COUT * KM
    SG1 = 32 * G1FIX
    SG2 = 1024
    NPLANES = 96

    xcat_d = nc.dram_tensor("xcat", [N // 2, 128], f16, kind="ExternalInput")
    dtab_d = nc.dram_tensor("dtab", [128, E // 128, 5], f16, kind="ExternalInput")
    bwd_d = nc.dram_tensor("bwd", [128, PCOLS, 2], f32, kind="ExternalInput")
    wfl_d = nc.dram_tensor("wfl", [CIN, OJ], f32, kind="ExternalInput")
    dtt_d = nc.dram_tensor("dt_t", [NUM_PTS, KM], f32, kind="ExternalInput")
    xidx1_d = nc.dram_tensor("xidx1", [128, SG1 * 128 // 16], i16, kind="ExternalInput")
    tab1_d = nc.dram_tensor("tab1", [128, 8 * SG1], f16, kind="ExternalInput")
    rotm_d = nc.dram_tensor("rotm", [2 * 128, 128], f16, kind="ExternalInput")
    yidx2_d = nc.dram_tensor("yidx2", [128, SG2 * 128 // 16], i16, kind="ExternalInput")
    tab2_d = nc.dram_tensor("tab2", [128, 8 * SG2], f16, kind="ExternalInput")
    iota_d = nc.dram_tensor("iota", [128, 128 * 34], f16, kind="ExternalInput")
    out_d = nc.dram_tensor("out", [N // 2, 64], f32, kind="ExternalOutput")
    ycat_d = nc.dram_tensor("ycat_tbl", [NUM_PTS, 64], f32, kind="Internal")

    with tile.TileContext(nc) as tc:
        with tc.tile_pool(name="consts", bufs=1) as cp:
            ident = cp.tile([128, 128], f32)
            masks.make_identity(nc, ident[:])
            nc.gpsimd.load_library(library_config.mlp)

            wfl = cp.tile([CIN, OJ], f32)
            nc.sync.dma_start(wfl[:], wfl_d[:])
            bwd = cp.tile([128, PCOLS * 2], f32)
            bwd3 = bwd[:].rearrange("p (q t) -> p q t", t=2)
            nc.sync.dma_start(bwd3, bwd_d[:])
            iota = cp.tile([128, 128 * G1FIX], f16)
            nc.sync.dma_start(iota[:], iota_d[:])
            rotm = cp.tile([128, 2 * 128], f16)
            nc.sync.dma_start(rotm[:].rearrange("p (c j) -> p c j", c=2),
                              rotm_d.ap().rearrange("(c p) j -> p c j", p=128))
            tab1 = cp.tile([128, 8 * SG1], f16)
            nc.sync.dma_start(tab1[:], tab1_d[:])
            tab13 = tab1[:].rearrange("p (t s) -> p t s", t=8)
            xi1 = cp.tile([128, SG1 * 8], i16)
            nc.sync.dma_start(xi1[:], xidx1_d[:])


            # ---------- dense pass: rnorm planes (cp pool: no SBUF reuse
            # so sigma1 gathers are not blocked behind this) ----------
            rnt = cp.tile([128, NPLANES], f16)
            with tc.tile_pool(name="dpsum", bufs=1, space="PSUM") as dq:
                JD = E // 128
                dtab = cp.tile([128, JD * 5], f16)
                dt3 = dtab[:].rearrange("p (j t) -> p j t", t=5)
                nc.sync.dma_start(dt3, dtab_d[:])
                dd0 = cp.tile([128, JD], f32)
                dd1 = cp.tile([128, JD], f32)
                nc.vector.tensor_tensor(dd0[:], dt3[:, :, 0], dt3[:, :, 3], op=subtract)
                nc.vector.tensor_tensor(dd0[:], dd0[:], dd0[:], op=mult)
                nc.vector.tensor_tensor(dd1[:], dt3[:, :, 1], dt3[:, :, 4], op=subtract)
                nc.vector.tensor_tensor(dd1[:], dd1[:], dd1[:], op=mult)
                d0k = dd0[:].rearrange("p (q k) -> p q k", k=K)
                d1k = dd1[:].rearrange("p (q k) -> p q k", k=K)
                nc.vector.tensor_tensor(d0k, d0k,
                                        bwd3[:, :, 0].broadcast_to((128, PCOLS, K)),
                                        op=mult)
                nc.vector.tensor_tensor(d1k, d1k,
                                        bwd3[:, :, 1].broadcast_to((128, PCOLS, K)),
                                        op=mult)
                nc.vector.tensor_tensor(dd0[:], dd0[:], dd1[:], op=add)
                nc.scalar.activation(dd1[:], dd0[:], Exp, scale=-1.0)
                nc.vector.tensor_tensor(dd1[:], dd1[:], dt3[:, :, 2], op=mult)
                nc.vector.tensor_tensor(dd1[:], dd1[:], dd1[:], op=mult)
                nsq = cp.tile([128, PCOLS], f32)
                nc.vector.reduce_sum(nsq[:].unsqueeze(2),
                                     dd1[:].rearrange("p (q k) -> p q k", k=K),
                                     axis=X)
                nc.scalar.activation(nsq[:], nsq[:],
                                     mybir.ActivationFunctionType.Sqrt)
                nc.vector.tensor_scalar_add(nsq[:], nsq[:], 1e-5)
                nc.vector.reciprocal(nsq[:], nsq[:])
                nc.vector.tensor_copy(rnt[:, 0:32], nsq[:])
                for ci in range(2):
                    rp = dq.tile([128, 32], f32, tag="rp")
                    nc.tensor.matmul(rp[:], rotm[:, ci * 128:(ci + 1) * 128],
                                     rnt[:, 0:32], start=True, stop=True)
                    nc.vector.tensor_copy(rnt[:, 32 + 32 * ci:64 + 32 * ci], rp[:])

            # ---------- rn_all: 96-plane select (4 chunks so sigma1's early
            # blocks unblock before the whole select finishes) ----------
            rn_all = cp.tile([128, SG1], f16)
            rtmp = cp.tile([128, SG1], f16)
            nc.vector.memset(rn_all[:], 0.0)
            prow = tab13[:, 7, :]
            RC = SG1 // 4
            for r in range(4):
                cs = slice(r * RC, (r + 1) * RC)
                for j in range(NPLANES):
                    nc.vector.scalar_tensor_tensor(
                        rtmp[:, cs], prow[:, cs], float(j),
                        rnt[:, j:j + 1].broadcast_to((128, RC)),
                        op0=is_equal, op1=mult)
                    nc.vector.tensor_tensor(rn_all[:, cs], rn_all[:, cs],
                                            rtmp[:, cs], op=add)

            # ---------- sigma1 + fused phase C ----------
            xhT = cp.tile([CIN, NUM_PTS], f32)
            with tc.tile_pool(name="ph1", bufs=3) as p1, \
                    tc.tile_pool(name="ps1", bufs=2, space="PSUM") as q1:
                for b in range(32):
                    sl = slice(b * G1FIX, (b + 1) * G1FIX)
                    gx = p1.tile([128, G1FIX * 128], f16, tag="gx", bufs=5)
                    gx3 = gx[:].rearrange("p (g e) -> p g e", e=128)
                    nc.gpsimd.dma_gather(
                        gx3, xcat_d[:],
                        xi1[:, b * G1FIX * 8:(b + 1) * G1FIX * 8],
                        G1FIX * 128, G1FIX * 128, 128,
                        elem_step=128, single_packet=False,
                        queue_num=b % 4)
                    me = tab13[:, 4, sl]
                    mo = tab13[:, 5, sl]
                    # grid/gw of the edge's node via parity select
                    ge = p1.tile([128, G1FIX * 3], f32, tag="ge")
                    ge3 = ge[:].rearrange("p (g t) -> p g t", t=3)
                    t0 = p1.tile([128, G1FIX * 3], f32, tag="t0")
                    t03 = t0[:].rearrange("p (g t) -> p g t", t=3)
                    nc.vector.tensor_tensor(
                        ge3, gx3[:, :, 32:35],
                        me.unsqueeze(2).broadcast_to((128, G1FIX, 3)), op=mult)
                    nc.vector.tensor_tensor(
                        t03, gx3[:, :, 96:99],
                        mo.unsqueeze(2).broadcast_to((128, G1FIX, 3)), op=mult)
                    nc.vector.tensor_tensor(ge3, ge3, t03, op=add)
                    dd = p1.tile([128, G1FIX * 2], f32, tag="dd")
                    dd3 = dd[:].rearrange("p (g t) -> p g t", t=2)
                    nc.vector.tensor_tensor(
                        dd3, ge3[:, :, 0:2],
                        tab13[:, 0:2, sl].rearrange("p t s -> p s t"), op=subtract)
                    nc.vector.tensor_tensor(dd3, dd3, dd3, op=mult)
                    nc.vector.tensor_tensor(
                        dd3, dd3,
                        tab13[:, 2:4, sl].rearrange("p t s -> p s t"), op=mult)
                    ga = p1.tile([128, G1FIX], f32, tag="ga")
                    nc.vector.tensor_tensor(ga[:], dd3[:, :, 0], dd3[:, :, 1],
                                            op=add)
                    nc.scalar.activation(ga[:], ga[:], Exp, scale=-1.0)
                    nc.vector.tensor_tensor(ga[:], ga[:], ge3[:, :, 2], op=mult)
                    nc.vector.tensor_tensor(ga[:], ga[:], rn_all[:, sl], op=mult)
                    wlo = p1.tile([128, G1FIX], f32, tag="wlo")
                    whi = p1.tile([128, G1FIX], f32, tag="whi")
                    nc.vector.tensor_tensor(wlo[:], ga[:], me, op=mult)
                    nc.vector.tensor_tensor(whi[:], ga[:], mo, op=mult)
                    v1 = p1.tile([128, G1FIX * CIN], f16, tag="v1")
                    v13 = v1[:].rearrange("p (g e) -> p g e", e=CIN)
                    t1 = p1.tile([128, G1FIX * CIN], f16, tag="t1")
                    t13 = t1[:].rearrange("p (g e) -> p g e", e=CIN)
                    nc.vector.tensor_tensor(
                        v13, gx3[:, :, 0:CIN],
                        wlo[:].unsqueeze(2).broadcast_to((128, G1FIX, CIN)),
                        op=mult)
                    nc.vector.tensor_tensor(
                        t13, gx3[:, :, 64:64 + CIN],
                        whi[:].unsqueeze(2).broadcast_to((128, G1FIX, CIN)),
                        op=mult)
                    nc.vector.tensor_tensor(v13, v13, t13, op=add)
                    oh = p1.tile([128, G1FIX * 128], f16, tag="oh")
                    oh3 = oh[:].rearrange("p (g e) -> p g e", e=128)
                    nc.vector.tensor_tensor(
                        oh3,
                        tab13[:, 6, sl].unsqueeze(2).broadcast_to((128, G1FIX, 128)),
                        iota[:].rearrange("p (g e) -> p g e", e=128),
                        op=is_equal)
                    ps = q1.tile([CIN, 128], f32, tag="pxh")
                    for g in range(G1FIX):
                        nc.tensor.matmul(ps[:], v13[:, g, :], oh3[:, g, :],
                                         start=(g == 0), stop=(g == G1FIX - 1))
                    nc.vector.tensor_copy(xhT[:, b * 128:(b + 1) * 128], ps[:])
                    # fused phase C for this 128-target tile
                    o1p = q1.tile([128, OJ], f32, tag="o1p")
                    nc.tensor.matmul(o1p[:], xhT[:, b * 128:(b + 1) * 128],
                                     wfl[:], start=True, stop=True)
                    dtt = p1.tile([128, KM], f32, tag="dtt")
                    nc.sync.dma_start(dtt[:], dtt_d[b * 128:(b + 1) * 128, :])
                    o1 = p1.tile([128, OJ], f32, tag="o1")
                    nc.vector.tensor_tensor(
                        o1[:].rearrange("p (o j) -> p o j", j=KM),
                        o1p[:].rearrange("p (o j) -> p o j", j=KM),
                        dtt[:].unsqueeze(1).broadcast_to((128, COUT, KM)),
                        op=mult)
                    yrow = p1.tile([128, 64], f32, tag="yrow")
                    nc.vector.reduce_sum(
                        yrow[:, 0:COUT].unsqueeze(2),
                        o1[:].rearrange("p (o j) -> p o j", j=KM), axis=X)
                    nc.sync.dma_start(
                        ycat_d.ap()[b * 128:(b + 1) * 128, 0:COUT], yrow[:, 0:COUT])

            # ---------- sigma2 ----------
            NCH = 256 // S2CHUNK          # chunks
            GC = S2CHUNK * 4              # group-columns per chunk (G2FIX=4)
            with tc.tile_pool(name="ph2", bufs=2) as p2, \
                    tc.tile_pool(name="ps2", bufs=2, space="PSUM") as q2:
                tab2 = p2.tile([128, 8 * SG2], f16, tag="tab2", bufs=1)
                nc.sync.dma_start(tab2[:], tab2_d[:])
                tab23 = tab2[:].rearrange("p (t s) -> p t s", t=8)
                yi2 = p2.tile([128, SG2 * 8], i16, tag="yi2", bufs=1)
                nc.sync.dma_start(yi2[:], yidx2_d[:])
                for c in range(NCH):
                    s0 = c * GC           # first group-col of chunk
                    sl = slice(s0, s0 + GC)
                    gy = p2.tile([128, GC * 64], f32, tag="gy", bufs=4)
                    gy3 = gy[:].rearrange("p (g e) -> p g e", e=64)
                    nc.gpsimd.dma_gather(
                        gy3, ycat_d[:],
                        yi2[:, s0 * 8:(s0 + GC) * 8],
                        GC * 128, GC * 128, 64,
                        elem_step=64, single_packet=False,
                        queue_num=c % 4)
                    dd = p2.tile([128, GC * 2], f32, tag="dd2")
                    dd3 = dd[:].rearrange("p (g t) -> p g t", t=2)
                    nc.vector.tensor_tensor(
                        dd3, tab23[:, 0:2, sl].rearrange("p t s -> p s t"),
                        tab23[:, 2:4, sl].rearrange("p t s -> p s t"), op=subtract)
                    nc.vector.tensor_tensor(dd3, dd3, dd3, op=mult)
                    nc.vector.tensor_tensor(
                        dd3, dd3,
                        tab23[:, 4:6, sl].rearrange("p t s -> p s t"), op=mult)
                    ga = p2.tile([128, GC], f32, tag="ga2")
                    nc.vector.tensor_tensor(ga[:], dd3[:, :, 0], dd3[:, :, 1],
                                            op=add)
                    nc.scalar.activation(ga[:], ga[:], Exp, scale=-1.0)
                    me2 = tab23[:, 7, sl]
                    gme = p2.tile([128, GC], f32, tag="gme")
                    gmo = p2.tile([128, GC], f32, tag="gmo")
                    nc.vector.tensor_tensor(gme[:], ga[:], me2, op=mult)
                    nc.vector.tensor_tensor(gmo[:], ga[:], gme[:], op=subtract)
                    v2e = p2.tile([128, GC * 32], f16, tag="v2e")
                    v2e3 = v2e[:].rearrange("p (g e) -> p g e", e=32)
                    v2o = p2.tile([128, GC * 32], f16, tag="v2o")
                    v2o3 = v2o[:].rearrange("p (g e) -> p g e", e=32)
                    nc.vector.tensor_tensor(
                        v2e3, gy3[:, :, 0:32],
                        gme[:].unsqueeze(2).broadcast_to((128, GC, 32)), op=mult)
                    nc.vector.tensor_tensor(
                        v2o3, gy3[:, :, 0:32],
                        gmo[:].unsqueeze(2).broadcast_to((128, GC, 32)), op=mult)
                    oh2 = p2.tile([128, GC * 128], f16, tag="oh2")
                    oh23 = oh2[:].rearrange("p (g e) -> p g e", e=128)
                    nc.vector.tensor_tensor(
                        oh23,
                        tab23[:, 6, sl].unsqueeze(2).broadcast_to((128, GC, 128)),
                        iota[:, :GC * 128].rearrange("p (g e) -> p g e", e=128),
                        op=is_equal)
                    ob = p2.tile([128, S2CHUNK * 64], f32, tag="ob")
                    ob3 = ob[:].rearrange("p (k e) -> p k e", e=64)
                    for k in range(S2CHUNK):
                        po = q2.tile([128, 64], f32, tag="po")
                        po3 = po[:].rearrange("p (h e) -> p h e", e=32)
                        for g in range(4):
                            gc = 4 * k + g
                            nc.tensor.matmul(po3[:, 0, :], oh23[:, gc, :],
                                             v2e3[:, gc, :],
                                             start=(g == 0), stop=(g == 3))
                        for g in range(4):
                            gc = 4 * k + g
                            nc.tensor.matmul(po3[:, 1, :], oh23[:, gc, :],
                                             v2o3[:, gc, :],
                                             start=(g == 0), stop=(g == 3))
                        nc.vector.tensor_copy(ob3[:, k, :], po[:])
                    nc.sync.dma_start(
                        out_d.ap()[c * S2CHUNK * 128:(c + 1) * S2CHUNK * 128, :]
                        .rearrange("(k p) e -> p k e", p=128),
                        ob3)
    return nc


def make_in_maps(cfg, x, grid, grid_weight, edge_grid, edge_Gauss, basepts,
                 base_weight, D, weights):
    maps, invs = [], []
    for b in range(x.shape[0]):
        m, inv2 = host_prep(cfg, x[b], grid[b], grid_weight[b], edge_grid[b],
                            edge_Gauss[b], basepts, base_weight, D, weights)
        maps.append(m)
        invs.append(inv2)
    return maps, invs


def finish(cfg, out_tbl, p_newrow):
    # device row p_newrow[pair] holds pair's output
    o = out_tbl[p_newrow]
    return np.ascontiguousarray(
        o.reshape(cfg["N"], 32)[:, :cfg["COUT"]].T)


_BUILT = {}


def _get_nc():
    if "nc" not in _BUILT:
        nc = bacc.Bacc("TRN2", target_bir_lowering=False,
                       dynamic_dma_scratch_size=32768,
                       num_swdge_queues=4)
        build(nc, CFG)
        nc.compile()
        _BUILT["nc"] = nc
    return _BUILT["nc"]


def kernel(x, grid, grid_weight, edge_grid, edge_Gauss, basepts, base_weight,
           D, weights, _trace=False):
    cfg = CFG
    in_maps, invs = make_in_maps(
        cfg, np.asarray(x, np.float32), np.asarray(grid),
        np.asarray(grid_weight), np.asarray(edge_grid),
        np.asarray(edge_Gauss), np.asarray(basepts),
        np.asarray(base_weight), np.asarray(D), np.asarray(weights))
    nc = _get_nc()
    res = bass_utils.run_bass_kernel_spmd(
        nc, in_maps, core_ids=list(range(x.shape[0])), trace=_trace)
    out = np.stack([finish(cfg, res.results[b]["out"], invs[b])
                    for b in range(x.shape[0])])
    kernel.last_result = res
    return out
